# revision 25
# baseline (speedup 1.0000x reference)
# MoE EnhancedGatedFusion kernel for 8x TRN2 NeuronCores (expert-parallel).
#
# Decomposition:
#   host : router logits -> top2 -> softmax gates -> dispatch by expert
#   L1   : per-core (expert e): H_T[d_out, n] = silu(We[e].T-contract @ XT + be[e])
#          (ungated; fp8e4 DoubleRow matmuls, fp32 PSUM, bf16 H output)
#   host : combine CT = g1*A + g2*B during the token un-shuffle (the
#          "all-to-all return" glue), downcast once to bf16; tokens past the
#          per-expert capacity (CAP) are the standard MoE overflow case and
#          are evaluated on host in fp32 during the combine.
#   L2   : per-core (1024 tokens): out = CT.T @ Wo; y = XIN + out (XIN =
#          x_shard + bo folded on host, fp32); RMS-norm * norm_w.
#
# L1 matmuls run fp8e4 (e4m3) in DoubleRow perf mode: each instruction
# contracts 256 rows (two 128-deep k-groups) at 1 output column/cycle --
# measured 1.96x the bf16 MAC rate on hardware.  Weights are prescaled by
# 64 on host to clear the e4m3 subnormal range (std 0.022 -> 1.4) and the
# 1/64 is folded into the silu activation's input scale.  End-to-end rel
# err of the fp8 L1 + bf16 L2 pipeline is ~1.6e-2 (gate: 2e-2).
# L2 stays bf16: fp8 there would push total error past the gate.
import sys
import types

sys.path.insert(0, "/opt/trn_rl_repo")

import numpy as np
import ml_dtypes

BF16 = np.dtype(ml_dtypes.bfloat16)


def _install_ntff_hook():
    # antenv.axon_hooks is missing in this image; shim it so
    # run_bass_kernel_spmd(trace=True) can drive NTFF profiling.
    if "antenv.axon_hooks" in sys.modules:
        return
    try:
        from trn_agent_boot.trn_boot import _ntff_profile_via_ctypes

        hook = _ntff_profile_via_ctypes("/opt/axon/libaxon_pjrt.so")
    except Exception:
        hook = None
    mod = types.ModuleType("antenv.axon_hooks")
    mod.get_axon_ntff_profile_hook = lambda: hook
    mod.set_axon_ntff_profile_hook = lambda h: None
    sys.modules["antenv.axon_hooks"] = mod


_install_ntff_hook()

import concourse.bacc as bacc
import concourse.bass as bass
import concourse.tile as tile
from concourse import mybir
from concourse.bass_utils import run_bass_kernel_spmd

F32 = mybir.dt.float32
BF = mybir.dt.bfloat16
F8 = mybir.dt.float8e4
F8NP = np.dtype(ml_dtypes.float8_e4m3)  # TRN fp8e4: max normal 240
P = 128
NCORE = 8
CAP = 2048      # per-expert token capacity on device; overflow runs on host
WSCALE = 64.0   # We prescale before fp8 cast (undone in the silu's scale)


def _chunks(total, size):
    out = []
    o = 0
    while o < total:
        out.append((o, min(size, total - o)))
        o += size
    return out


def build_l1(D, cap):
    """Per-core expert FFN: H[d_out, n] = silu((1/WSCALE)*sum_k W[k,d_out]*XT[k,n] + be[d_out]).

    fp8e4 DoubleRow matmuls: XT is host-pretiled fp8 [C, P, K2, 2, 512]
    (k-tiles paired into DoubleRow groups) and W is fp8 [P, F, K2, 2, P]
    (partition-major so multi-f-tile slabs DMA as one transfer with long
    per-partition runs), prescaled by WSCALE on host.  W is fully
    SBUF-resident (4.2MB), XT streams through once.  H output is bf16,
    written in 4-f-tile batches [C, FQ, P, 4, 512] so each store DMA has
    4KB per-partition runs.

    DMA discipline: every dma_start costs ~0.6us of serial config time on
    the issuing engine's sequencer, so transfers are few and large.  All
    loads ride the SP (sync) HWDGE queue (no compute there); H stores ride
    the Activation queue, whose configs interleave naturally with the
    silus that produce them.
    """
    K2 = D // P // 2    # DoubleRow k-pairs
    F = D // P          # feat (d_out) tiles
    FQ = F // 4         # H store batches
    C = cap // 512      # 512-col chunks
    assert cap % 512 == 0
    nc = bacc.Bacc("TRN2", target_bir_lowering=False, debug=False)
    XT = nc.dram_tensor("XT", [C, P, K2, 2, 512], F8, kind="ExternalInput")
    W = nc.dram_tensor("W", [P, F, K2, 2, P], F8, kind="ExternalInput")
    BE = nc.dram_tensor("BE", [D], F32, kind="ExternalInput")
    H = nc.dram_tensor("H", [C, FQ, P, 4, 512], BF, kind="ExternalOutput")
    DR = mybir.MatmulPerfMode.DoubleRow

    with tile.TileContext(nc) as tc:
        with (
            tc.tile_pool(name="consts", bufs=1) as consts,
            tc.tile_pool(name="xt", bufs=3) as xtp,
            tc.tile_pool(name="hout", bufs=3) as hp,
            tc.tile_pool(name="ps", bufs=4, space="PSUM") as psp,
        ):
            # PE warm-up: ~48 tiny matmuls during the DMA lead-in keep the
            # HAM activity window busy so real matmuls start at 2.4GHz.
            warm = consts.tile([P, 64], BF)
            nc.vector.memset(warm[:], 1.0)
            wps = psp.tile([P, 512], F32, tag="ps", name="ps")
            for _ in range(48):
                nc.tensor.matmul(wps[0:64, 0:64], lhsT=warm[:, 0:64],
                                 rhs=warm[:], start=True, stop=True)

            # sync-queue load order = consumption order: W0/XT0 k-pieces
            # first so the f=0 k-loop starts ~3us in, then W slabs paced
            # against the 1.8us/f-tile burn rate, XT chunks behind.
            w_all = consts.tile([P, F, K2, 2, P], F8)
            xt_tiles = {}
            xt_tiles[0] = xtp.tile([P, K2, 2, 512], F8, tag="xt", name="xt")
            nc.sync.dma_start(w_all[:, 0, 0:2], W[:, 0, 0:2])
            nc.sync.dma_start(xt_tiles[0][:, 0:2], XT[0, :, 0:2])
            nc.sync.dma_start(w_all[:, 0, 2 : K2], W[:, 0, 2 : K2])
            nc.sync.dma_start(xt_tiles[0][:, 2:4], XT[0, :, 2:4])
            nc.sync.dma_start(xt_tiles[0][:, 4 : K2], XT[0, :, 4 : K2])
            be_sb = consts.tile([P, F], F32)
            nc.sync.dma_start(be_sb[:], BE[:].rearrange("(f p) -> p f", p=P))
            nc.sync.dma_start(w_all[:, 1:3], W[:, 1:3])
            nc.sync.dma_start(w_all[:, 3:7], W[:, 3:7])
            nc.sync.dma_start(w_all[:, 7:11], W[:, 7:11])
            nc.sync.dma_start(w_all[:, 11:F], W[:, 11:F])
            for ci in range(1, C):
                xt_c = xtp.tile([P, K2, 2, 512], F8, tag="xt", name="xt")
                nc.sync.dma_start(xt_c[:], XT[ci])
                xt_tiles[ci] = xt_c

            for ci in range(C):
                xt_c = xt_tiles[ci]
                h4 = None
                for f in range(F):
                    if f % 4 == 0:
                        h4 = hp.tile([P, 4, 512], BF, tag="h4", name="h4")
                    ps = psp.tile([P, 512], F32, tag="ps", name="ps")
                    for k in range(K2):
                        nc.tensor.matmul(
                            ps[:],
                            lhsT=w_all[:, f, k],
                            rhs=xt_c[:, k],
                            start=(k == 0),
                            stop=(k == K2 - 1),
                            perf_mode=DR,
                        )
                    nc.scalar.activation(
                        h4[:, f % 4],
                        ps[:],
                        mybir.ActivationFunctionType.Silu,
                        bias=be_sb[:, f : f + 1],
                        scale=1.0 / WSCALE,
                    )
                    last_batch = ci == C - 1 and f // 4 == FQ - 1
                    if last_batch and f % 4 == 1:
                        # stream the final batch per-silu so the drain after
                        # the last silu is one f-tile on the idle sync queue
                        nc.scalar.dma_start(H[ci, f // 4, :, 0:2], h4[:, 0:2])
                    elif last_batch and f % 4 == 2:
                        nc.scalar.dma_start(H[ci, f // 4, :, 2:3], h4[:, 2:3])
                    elif last_batch and f % 4 == 3:
                        nc.sync.dma_start(H[ci, f // 4, :, 3:4], h4[:, 3:4])
                    elif f % 4 == 3:
                        nc.scalar.dma_start(H[ci, f // 4], h4[:])
    nc.compile()
    return nc


def build_l2(D, TPC, unit_nw, eps=1e-6):
    """Per-core combine + output proj + residual + RMS norm.

    Y[t, j] = nw[j] * (XIN[t,j] + sum_k CT[k,t]*Wo[k,j]) / rms(t)
    CT = g1*A + g2*B (host-combined, bf16); XIN = x_shard + bo (fp32).
    Y output is bf16 (host upcasts).

    The last n-chunk runs m-outer so each m's epilogue (RMS + scale +
    store) chains behind its own k-loop and overlaps the next m's
    matmuls; only m=M-1's epilogue trails the final matmul.

    unit_nw=True specializes norm_w == 1 (scale-by-rstd runs as a scalar
    engine activation; multiplying by 1 is exact) so the vector engine
    stays under the per-m tensor budget during the epilogue phase.
    """
    K = D // P
    M = TPC // P
    NC4 = D // 512
    KB = K // 4          # k-tiles bundled per DMA
    nc = bacc.Bacc("TRN2", target_bir_lowering=False, debug=False)
    # CT/WO are host-pretiled so every DMA reads long contiguous
    # per-partition runs (8KB / 4KB) — strided reads from the natural
    # [D, x] layout only sustain ~40% of DMA bandwidth and gate the lead-in.
    # DMA discipline: dma_start costs ~0.6us serial config time on the
    # issuing engine, so transfers are few and large.  CT + XIN ride the
    # SP (sync) queue, WO rides the Activation queue (its prelude configs
    # precede all Act compute in program order).
    CT = nc.dram_tensor("CT", [KB, P, 4, TPC], BF, kind="ExternalInput")
    XIN = nc.dram_tensor("XIN", [TPC, D], BF, kind="ExternalInput")
    WO = nc.dram_tensor("WO", [NC4, KB, P, 4, 512], BF, kind="ExternalInput")
    NW = nc.dram_tensor("NW", [D], F32, kind="ExternalInput")
    Y = nc.dram_tensor("Y", [TPC, D], BF, kind="ExternalOutput")

    XINr = XIN[:, :].rearrange("(m p) d -> p m d", p=P)

    with tile.TileContext(nc) as tc:
        with (
            tc.tile_pool(name="consts", bufs=1) as consts,
            tc.tile_pool(name="ct", bufs=1) as ctp,
            tc.tile_pool(name="wo", bufs=3) as wop,
            tc.tile_pool(name="yall", bufs=1) as yallp,
            tc.tile_pool(name="sq", bufs=3) as sqp,
            tc.tile_pool(name="xin", bufs=2) as xinp,
            tc.tile_pool(name="yn", bufs=2) as ynp,
            tc.tile_pool(name="ssm", bufs=1) as ssmp,
            tc.tile_pool(name="stat", bufs=4) as statp,
            tc.tile_pool(name="ps", bufs=1, space="PSUM") as psp,
        ):
            # CT as one SBUF-resident tile; first bundle k-split so the
            # n=0 k-loop starts after ~0.4MB.
            ct_all = ctp.tile([P, KB, 4, TPC], BF)
            # first k-tile split by m-range: the k=0/m=0 matmul only gates
            # on 64KB of CT + 128KB of WO.  CT's last bundle rides the
            # scalar queue (between wo-n0 and wo-n1) -- the n=0 k-loop
            # consumes 6MB in 17us, which needs both queues flat out.
            nc.sync.dma_start(ct_all[:, 0, 0:1, 0:256], CT[0, :, 0:1, 0:256])
            wo_cur = wop.tile([P, KB, 4, 512], BF, tag="wo", name="wo")
            WOr = [WO[n].rearrange("b p j x -> p b j x") for n in range(NC4)]
            nc.scalar.dma_start(wo_cur[:, 0, 0:1], WOr[0][:, 0, 0:1])
            nc.sync.dma_start(ct_all[:, 0, 0:1, 256:TPC], CT[0, :, 0:1, 256:TPC])
            nc.scalar.dma_start(wo_cur[:, 0, 1:4], WOr[0][:, 0, 1:4])
            nc.sync.dma_start(ct_all[:, 0, 1:4], CT[0, :, 1:4])
            nc.scalar.dma_start(wo_cur[:, 1:KB], WOr[0][:, 1:KB])
            nc.sync.dma_start(ct_all[:, 1], CT[1])
            nc.sync.dma_start(ct_all[:, 2], CT[2])
            nc.scalar.dma_start(ct_all[:, 3], CT[3])
            wo_nxt = wop.tile([P, KB, 4, 512], BF, tag="wo", name="wo")
            nc.scalar.dma_start(wo_nxt[:, 0:1], WOr[1][:, 0:1])
            nc.scalar.dma_start(wo_nxt[:, 1:KB], WOr[1][:, 1:KB])
            # y_all accumulator (fp32); the residual XIN streams in as
            # per-(m,n) bf16 slices added at psum-eviction time, so its
            # bytes never compete with the lead-in wo/ct stream.
            y_all = yallp.tile([P, M, D], F32)
            nw_sb = None
            if not unit_nw:
                nw_sb = consts.tile([P, D], F32)
                nwap = NW[:]
                nw_bcast = bass.AP(
                    tensor=nwap.tensor, offset=nwap.offset, ap=[[0, P]] + list(nwap.ap)
                )
                nc.sync.dma_start(nw_sb[:], nw_bcast)
            eps_sb = consts.tile([P, 1], F32)
            nc.vector.memset(eps_sb[:], eps)

            ssm_t = ssmp.tile([P, M], F32)
            ss_m = [ssm_t[:, m : m + 1] for m in range(M)]

            def stats(m, n, y_slice, first):
                # incremental RMS stats: ss_m[m] += sum(y_slice^2)
                ncols = y_slice.shape[-1]
                sq = sqp.tile([P, 1024], F32, tag="sq", name="sq")
                ssp = statp.tile([P, 1], F32, tag="ssp", name="ssp")
                nc.scalar.activation(
                    sq[:, :ncols],
                    y_slice,
                    mybir.ActivationFunctionType.Square,
                    accum_out=ssp[:],
                )
                if first:
                    nc.vector.tensor_copy(ss_m[m], ssp[:])
                else:
                    nc.vector.tensor_add(ss_m[m], ss_m[m], ssp[:])

            def epilogue(m):
                y_m = y_all[:, m, :]
                rms = statp.tile([P, 1], F32, tag="rms", name="rms")
                nc.scalar.activation(
                    rms[:],
                    ss_m[m],
                    mybir.ActivationFunctionType.Sqrt,
                    bias=eps_sb[:],
                    scale=1.0 / D,
                )
                rstd = statp.tile([P, 1], F32, tag="rstd", name="rstd")
                nc.vector.reciprocal(rstd[:], rms[:])
                yn = ynp.tile([P, D], BF, tag="yn", name="yn")
                if unit_nw and m == M - 1:
                    # final m: its scale trails the last matmul; pipeline
                    # 512-col pieces across scalar+vector+gpsimd and both
                    # HWDGE queues so the last Y write starts ASAP
                    for q in range(4):
                        qs = slice(q * 512, (q + 1) * 512)
                        if q == 0 or q == 3:
                            nc.scalar.activation(
                                yn[:, qs],
                                y_all[:, m, qs],
                                mybir.ActivationFunctionType.Identity,
                                bias=0.0,
                                scale=rstd[:],
                            )
                        elif q == 1:
                            nc.vector.tensor_scalar_mul(
                                yn[:, qs], y_all[:, m, qs], rstd[:]
                            )
                        else:
                            nc.gpsimd.tensor_scalar_mul(
                                yn[:, qs], y_all[:, m, qs], rstd[:]
                            )
                        eng = nc.sync if q % 2 == 0 else nc.scalar
                        eng.dma_start(Y[m * P : (m + 1) * P, qs], yn[:, qs])
                    return
                for h in range(1):
                    hs = slice(0, D)
                    if unit_nw:
                        # big scales run on the otherwise-idle gpsimd (Pool)
                        # engine so neither scalar nor vector backlogs
                        # behind the 3.4us/m tensor pace
                        nc.gpsimd.tensor_scalar_mul(
                            yn[:, hs], y_all[:, m, hs], rstd[:]
                        )
                    else:
                        nc.vector.scalar_tensor_tensor(
                            yn[:, hs],
                            y_all[:, m, hs],
                            rstd[:],
                            nw_sb[:, hs],
                            op0=mybir.AluOpType.mult,
                            op1=mybir.AluOpType.mult,
                        )
                    nc.sync.dma_start(Y[m * P : (m + 1) * P, hs], yn[:, hs])

            pss = [psp.tile([P, 512], F32, tag=f"ps{m}", name=f"ps{m}")
                   for m in range(M)]

            # PE warm-up during the DMA lead-in (see build_l1)
            warm = consts.tile([P, 64], BF)
            nc.vector.memset(warm[:], 1.0)
            for _ in range(48):
                nc.tensor.matmul(pss[0][0:64, 0:64], lhsT=warm[:, 0:64],
                                 rhs=warm[:], start=True, stop=True)

            xin_prev = None
            for n in range(NC4):
                n0 = n * 512
                # one batched XIN load per n-chunk
                xin_n = xinp.tile([P, M, 512], BF, tag="xin", name="xin")
                nc.sync.dma_start(xin_n[:], XINr[:, :, n0 : n0 + 512])
                if n + 2 < NC4:
                    wo_n2 = wop.tile([P, KB, 4, 512], BF, tag="wo", name="wo")
                    nc.scalar.dma_start(wo_n2[:], WOr[n + 2])
                if n == 0:
                    # k-outer for the first chunk: consumes each (wo, ct)
                    # bundle over 8 matmuls, pacing the k-loop to the DMA
                    # stream instead of stalling m=0 on the full 6MB.
                    for k in range(K):
                        for m in range(M):
                            nc.tensor.matmul(
                                pss[m][:],
                                lhsT=ct_all[:, k // 4, k % 4, m * P : (m + 1) * P],
                                rhs=wo_cur[:, k // 4, k % 4],
                                start=(k == 0),
                                stop=(k == K - 1),
                            )
                    for m in range(M):
                        # defer the n=0 residual add: evict psum with a
                        # plain copy so nothing here waits on XIN
                        nc.vector.tensor_copy(y_all[:, m, 0:512], pss[m][:])
                else:
                    # m-outer, k-inner: 16 consecutive matmuls accumulate
                    # into one PSUM bank before it's read (avoids psum-queue
                    # depth-cycling micro-idles).
                    for m in range(M):
                        for k in range(K):
                            nc.tensor.matmul(
                                pss[m][:],
                                lhsT=ct_all[:, k // 4, k % 4, m * P : (m + 1) * P],
                                rhs=wo_cur[:, k // 4, k % 4],
                                start=(k == 0),
                                stop=(k == K - 1),
                            )
                        y_slice = y_all[:, m, n0 : n0 + 512]
                        nc.vector.tensor_add(y_slice, xin_n[:, m], pss[m][:])
                        if n == 1:
                            # catch up n=0's deferred residual add and run
                            # stats over both chunks at once
                            nc.vector.tensor_add(
                                y_all[:, m, 0:512], y_all[:, m, 0:512],
                                xin_prev[:, m],
                            )
                            stats(m, n, y_all[:, m, 0:1024], first=True)
                        else:
                            stats(m, n, y_slice, first=False)
                        if n + 1 == NC4:
                            # chain each m's epilogue behind its own k-loop
                            # so only m=M-1's trails the final matmul
                            epilogue(m)
                xin_prev = xin_n
                if n + 1 < NC4:
                    wo_cur = wo_nxt
                    if n + 2 < NC4:
                        wo_nxt = wo_n2
    nc.compile()
    return nc


def host_dispatch(xf, Wr, br):
    """Router + top-2 + softmax gates + expert grouping. Returns dispatch info."""
    T, D = xf.shape
    E = Wr.shape[1]
    logits = xf @ Wr + br
    i1 = np.argmax(logits, axis=1)
    l2 = logits.copy()
    l2[np.arange(T), i1] = -np.inf
    i2 = np.argmax(l2, axis=1)
    v1 = logits[np.arange(T), i1]
    v2 = logits[np.arange(T), i2]
    e2 = np.exp(v2 - v1)
    g1 = (1.0 / (1.0 + e2)).astype(np.float32)
    g2 = (e2 / (1.0 + e2)).astype(np.float32)

    # flat slots (t,s) grouped by expert, stable by (token, slot)
    ee = np.stack([i1, i2], 1).ravel()          # [2T]
    gg = np.stack([g1, g2], 1).ravel()
    tt = np.repeat(np.arange(T), 2)
    order = np.argsort(ee, kind="stable")
    counts = np.bincount(ee, minlength=E)
    starts = np.concatenate([[0], np.cumsum(counts)[:-1]])
    rank = np.empty(2 * T, np.int64)
    rank[order] = np.arange(2 * T)
    pos = rank - starts[ee]                      # position within expert's list
    return dict(
        e1=i1, e2=i2, counts=counts, order=order, starts=starts,
        pos=pos.reshape(T, 2), tok=tt, gate=gg, g1=g1, g2=g2,
    )


def prep_l1_inputs(xf, d, We, be):
    """Per-expert L1 inputs: gathered+pretiled fp8 XT, fp8 W (x WSCALE),
    fp32 be.  Tokens past CAP (expert overflow) are evaluated here on host
    in fp32 and stashed in d["Hov"] for the combine.
    """
    T, D = xf.shape
    E = We.shape[0]
    K2 = D // P // 2
    F = D // P
    C = CAP // 512
    counts = d["counts"]
    We_f = np.asarray(We, np.float32)
    be_f = np.asarray(be, np.float32)
    in1 = []
    Hov = []
    for e in range(E):
        n_e = int(counts[e])
        sel = d["order"][d["starts"][e] : d["starts"][e] + n_e]
        toks = d["tok"][sel]
        n_dev = min(n_e, CAP)
        Xg = np.zeros((CAP, D), np.float32)
        Xg[:n_dev] = xf[toks[:n_dev]]
        # [C, P, K2, 2, n]: DoubleRow k-pair groups, contiguous 8KB
        # per-partition DMA runs
        XT_T = np.ascontiguousarray(
            Xg.astype(F8NP).reshape(C, 512, K2, 2, P).transpose(0, 4, 2, 3, 1)
        )
        # [P, F, K2, 2, P]: partition-major so f-tile slabs DMA as single
        # transfers with (4*f_span)KB per-partition runs
        W_T = np.ascontiguousarray(
            (We_f[e] * WSCALE).astype(F8NP)
            .reshape(K2, 2, P, F, P).transpose(2, 3, 0, 1, 4)
        )
        in1.append({"XT": XT_T, "W": W_T, "BE": be_f[e]})
        if n_e > CAP:
            Xov = xf[toks[CAP:]]                        # [m, D] fp32
            pre = Xov @ We_f[e] + be_f[e]
            ho = (pre / (1.0 + np.exp(-pre))).astype(np.float32)
            Hov.append(ho.T)                            # [D, m]
        else:
            Hov.append(np.zeros((D, 0), np.float32))
    d["Hov"] = Hov
    return in1, CAP


def prep_l2_inputs(xf, d, H, Wo, bo, norm_w):
    """Per-core L2 inputs. CT = g1*A + g2*B combined on host (fp32 math,
    one bf16 downcast); XIN = x + bo in fp32."""
    T, D = xf.shape
    TPC = T // NCORE
    KB = D // P // 4
    NC4 = D // 512
    # pretile Wo into contiguous (n-chunk, k-bundle) blocks
    Wo_b = np.ascontiguousarray(
        np.asarray(Wo, np.float32)
        .reshape(KB, 4, P, NC4, 512)
        .transpose(3, 0, 2, 1, 4)
    ).astype(BF16)
    bo_f = np.asarray(bo, np.float32)
    nw_f = np.asarray(norm_w, np.float32)
    e1, e2, pos = d["e1"], d["e2"], d["pos"]
    g1, g2 = d["g1"], d["g2"]
    # device H (raw [C, FQ, P, 4, 512] batches -> [D, CAP], first CAP
    # slots) + host-computed overflow columns
    def unpack_h(h_raw):
        return np.ascontiguousarray(
            np.asarray(h_raw, np.float32).transpose(1, 3, 2, 0, 4).reshape(D, CAP)
        )
    Hfull = [np.concatenate([unpack_h(H[e]), d["Hov"][e]], axis=1)
             for e in range(H.shape[0])]
    in2 = []
    for c in range(NCORE):
        tl = np.arange(c * TPC, (c + 1) * TPC)
        CTf = np.empty((D, TPC), np.float32)
        BTf = np.empty((D, TPC), np.float32)
        for e in range(H.shape[0]):
            s1 = e1[tl] == e
            if s1.any():
                CTf[:, s1] = Hfull[e][:, pos[tl[s1], 0]]
            s2 = e2[tl] == e
            if s2.any():
                BTf[:, s2] = Hfull[e][:, pos[tl[s2], 1]]
        CTf = CTf * g1[tl][None, :] + BTf * g2[tl][None, :]
        CTt = np.ascontiguousarray(
            CTf.reshape(KB, 4, P, TPC).transpose(0, 2, 1, 3)
        ).astype(BF16)
        XIN = (xf[tl] + bo_f[None, :]).astype(BF16)
        in2.append({"CT": CTt, "XIN": XIN, "WO": Wo_b, "NW": nw_f})
    return in2


# ----------------------------------------------------------------------------
# Harness entry point: full (unsharded) inputs -> full output.
# ----------------------------------------------------------------------------
_L1_CACHE = {}
_L2_CACHE = {}


def kernel(x, Wr, br, We, be, Wo, bo, norm_w):
    B, S, D = x.shape
    E = We.shape[0]
    T = B * S
    TPC = T // NCORE
    xf = np.ascontiguousarray(np.asarray(x, np.float32).reshape(T, D))
    d = host_dispatch(xf, np.asarray(Wr, np.float32), np.asarray(br, np.float32))

    in1, Bcap = prep_l1_inputs(xf, d, We, be)
    if (D, Bcap) not in _L1_CACHE:
        _L1_CACHE[(D, Bcap)] = build_l1(D, Bcap)
    r1 = run_bass_kernel_spmd(_L1_CACHE[(D, Bcap)], in1, list(range(NCORE)))
    H = np.stack([r1.results[e]["H"] for e in range(E)])

    in2 = prep_l2_inputs(xf, d, H, Wo, bo, norm_w)
    unit_nw = bool(np.all(np.asarray(norm_w, np.float32) == 1.0))
    if (D, TPC, unit_nw) not in _L2_CACHE:
        _L2_CACHE[(D, TPC, unit_nw)] = build_l2(D, TPC, unit_nw)
    r2 = run_bass_kernel_spmd(_L2_CACHE[(D, TPC, unit_nw)], in2, list(range(NCORE)))
    Y = np.concatenate([r2.results[c]["Y"] for c in range(NCORE)], axis=0)
    return Y.reshape(B, S, D).astype(np.asarray(x).dtype)



# revision 26
# speedup vs baseline: 1.6935x; 1.6935x over previous
# MoE EnhancedGatedFusion kernel for 8x TRN2 NeuronCores (expert-parallel).
#
# Decomposition:
#   host : router logits -> top2 -> softmax gates -> dispatch by expert
#   L1   : per-core (expert e): H_T[d_out, n] = silu(We[e].T-contract @ XT + be[e])
#          (ungated; fp8e4 DoubleRow matmuls, fp32 PSUM, bf16 H output)
#   host : combine CT = g1*A + g2*B during the token un-shuffle (the
#          "all-to-all return" glue), downcast once to bf16; tokens past the
#          per-expert capacity (CAP) are the standard MoE overflow case and
#          are evaluated on host in fp32 during the combine.
#   L2   : per-core (1024 tokens): out = CT.T @ Wo; y = XIN + out (XIN =
#          x_shard + bo folded on host, fp32); RMS-norm * norm_w.
#
# L1 matmuls run fp8e4 (e4m3) in DoubleRow perf mode: each instruction
# contracts 256 rows (two 128-deep k-groups) at 1 output column/cycle --
# measured 1.96x the bf16 MAC rate on hardware.  Weights are prescaled by
# 64 on host to clear the e4m3 subnormal range (std 0.022 -> 1.4) and the
# 1/64 is folded into the silu activation's input scale.  End-to-end rel
# err of the fp8 L1 + bf16 L2 pipeline is ~1.6e-2 (gate: 2e-2).
# L2 stays bf16: fp8 there would push total error past the gate.
import sys
import types

sys.path.insert(0, "/opt/trn_rl_repo")

import numpy as np
import ml_dtypes

BF16 = np.dtype(ml_dtypes.bfloat16)


def _install_ntff_hook():
    # antenv.axon_hooks is missing in this image; shim it so
    # run_bass_kernel_spmd(trace=True) can drive NTFF profiling.
    if "antenv.axon_hooks" in sys.modules:
        return
    try:
        from trn_agent_boot.trn_boot import _ntff_profile_via_ctypes

        hook = _ntff_profile_via_ctypes("/opt/axon/libaxon_pjrt.so")
    except Exception:
        hook = None
    mod = types.ModuleType("antenv.axon_hooks")
    mod.get_axon_ntff_profile_hook = lambda: hook
    mod.set_axon_ntff_profile_hook = lambda h: None
    sys.modules["antenv.axon_hooks"] = mod


_install_ntff_hook()

import concourse.bacc as bacc
import concourse.bass as bass
import concourse.tile as tile
from concourse import mybir
from concourse.bass_utils import run_bass_kernel_spmd

F32 = mybir.dt.float32
BF = mybir.dt.bfloat16
F8 = mybir.dt.float8e4
F8NP = np.dtype(ml_dtypes.float8_e4m3)  # TRN fp8e4: max normal 240
P = 128
NCORE = 8
CAP = 2048      # per-expert token capacity on device; overflow runs on host
WSCALE = 64.0   # We prescale before fp8 cast (undone in the silu's scale)


def _chunks(total, size):
    out = []
    o = 0
    while o < total:
        out.append((o, min(size, total - o)))
        o += size
    return out


def build_l1(D, cap):
    """Per-core expert FFN: H[d_out, n] = silu((1/WSCALE)*sum_k W[k,d_out]*XT[k,n] + be[d_out]).

    fp8e4 DoubleRow matmuls: XT is host-pretiled fp8 [C, P, K2, 2, 512]
    (k-tiles paired into DoubleRow groups) and W is fp8 [P, F, K2, 2, P]
    (partition-major so multi-f-tile slabs DMA as one transfer with long
    per-partition runs), prescaled by WSCALE on host.  W is fully
    SBUF-resident (4.2MB), XT streams through once.  H output is bf16,
    written in 4-f-tile batches [C, FQ, P, 4, 512] so each store DMA has
    4KB per-partition runs.

    DMA discipline: every dma_start costs ~0.6us of serial config time on
    the issuing engine's sequencer, so transfers are few and large.  All
    loads ride the SP (sync) HWDGE queue (no compute there); H stores ride
    the Activation queue, whose configs interleave naturally with the
    silus that produce them.
    """
    K2 = D // P // 2    # DoubleRow k-pairs
    F = D // P          # feat (d_out) tiles
    FQ = F // 4         # H store batches
    C = cap // 512      # 512-col chunks
    assert cap % 512 == 0
    nc = bacc.Bacc("TRN2", target_bir_lowering=False, debug=False)
    XT = nc.dram_tensor("XT", [C, P, K2, 2, 512], F8, kind="ExternalInput")
    W = nc.dram_tensor("W", [P, F, K2, 2, P], F8, kind="ExternalInput")
    BE = nc.dram_tensor("BE", [D], F32, kind="ExternalInput")
    H = nc.dram_tensor("H", [C, FQ, P, 4, 512], BF, kind="ExternalOutput")
    DR = mybir.MatmulPerfMode.DoubleRow

    with tile.TileContext(nc) as tc:
        with (
            tc.tile_pool(name="consts", bufs=1) as consts,
            tc.tile_pool(name="xt", bufs=3) as xtp,
            tc.tile_pool(name="hout", bufs=3) as hp,
            tc.tile_pool(name="ps", bufs=4, space="PSUM") as psp,
        ):
            # PE warm-up: ~48 tiny matmuls during the DMA lead-in keep the
            # HAM activity window busy so real matmuls start at 2.4GHz.
            warm = consts.tile([P, 64], BF)
            nc.vector.memset(warm[:], 1.0)
            wps = psp.tile([P, 512], F32, tag="ps", name="ps")
            for _ in range(48):
                nc.tensor.matmul(wps[0:64, 0:64], lhsT=warm[:, 0:64],
                                 rhs=warm[:], start=True, stop=True)

            # sync-queue load order = consumption order: W0/XT0 k-pieces
            # first so the f=0 k-loop starts ~3us in, then W slabs paced
            # against the 1.8us/f-tile burn rate, XT chunks behind.
            w_all = consts.tile([P, F, K2, 2, P], F8)
            xt_tiles = {}
            xt_tiles[0] = xtp.tile([P, K2, 2, 512], F8, tag="xt", name="xt")
            nc.sync.dma_start(w_all[:, 0, 0:2], W[:, 0, 0:2])
            nc.sync.dma_start(xt_tiles[0][:, 0:2], XT[0, :, 0:2])
            nc.sync.dma_start(w_all[:, 0, 2 : K2], W[:, 0, 2 : K2])
            nc.sync.dma_start(xt_tiles[0][:, 2:4], XT[0, :, 2:4])
            nc.sync.dma_start(xt_tiles[0][:, 4 : K2], XT[0, :, 4 : K2])
            be_sb = consts.tile([P, F], F32)
            nc.sync.dma_start(be_sb[:], BE[:].rearrange("(f p) -> p f", p=P))
            nc.sync.dma_start(w_all[:, 1:3], W[:, 1:3])
            nc.sync.dma_start(w_all[:, 3:7], W[:, 3:7])
            nc.sync.dma_start(w_all[:, 7:11], W[:, 7:11])
            nc.sync.dma_start(w_all[:, 11:F], W[:, 11:F])
            for ci in range(1, C):
                xt_c = xtp.tile([P, K2, 2, 512], F8, tag="xt", name="xt")
                nc.sync.dma_start(xt_c[:], XT[ci])
                xt_tiles[ci] = xt_c

            for ci in range(C):
                xt_c = xt_tiles[ci]
                h4 = None
                for f in range(F):
                    if f % 4 == 0:
                        h4 = hp.tile([P, 4, 512], BF, tag="h4", name="h4")
                    ps = psp.tile([P, 512], F32, tag="ps", name="ps")
                    for k in range(K2):
                        nc.tensor.matmul(
                            ps[:],
                            lhsT=w_all[:, f, k],
                            rhs=xt_c[:, k],
                            start=(k == 0),
                            stop=(k == K2 - 1),
                            perf_mode=DR,
                        )
                    nc.scalar.activation(
                        h4[:, f % 4],
                        ps[:],
                        mybir.ActivationFunctionType.Silu,
                        bias=be_sb[:, f : f + 1],
                        scale=1.0 / WSCALE,
                    )
                    last_batch = ci == C - 1 and f // 4 == FQ - 1
                    if last_batch and f % 4 == 1:
                        # stream the final batch per-silu so the drain after
                        # the last silu is one f-tile on the idle sync queue
                        nc.scalar.dma_start(H[ci, f // 4, :, 0:2], h4[:, 0:2])
                    elif last_batch and f % 4 == 2:
                        nc.scalar.dma_start(H[ci, f // 4, :, 2:3], h4[:, 2:3])
                    elif last_batch and f % 4 == 3:
                        nc.sync.dma_start(H[ci, f // 4, :, 3:4], h4[:, 3:4])
                    elif f % 4 == 3:
                        nc.scalar.dma_start(H[ci, f // 4], h4[:])
    nc.compile()
    return nc


def build_l2(D, TPC, unit_nw, eps=1e-6):
    """Per-core combine + output proj + residual + RMS norm.

    Y[t, j] = nw[j] * (XIN[t,j] + sum_k CT[k,t]*Wo[k,j]) / rms(t)
    CT = g1*A + g2*B (host-combined, bf16); XIN = x_shard + bo (fp32).
    Y output is bf16 (host upcasts).

    The last n-chunk runs m-outer so each m's epilogue (RMS + scale +
    store) chains behind its own k-loop and overlaps the next m's
    matmuls; only m=M-1's epilogue trails the final matmul.

    unit_nw=True specializes norm_w == 1 (scale-by-rstd runs as a scalar
    engine activation; multiplying by 1 is exact) so the vector engine
    stays under the per-m tensor budget during the epilogue phase.
    """
    K = D // P
    M = TPC // P
    NC4 = D // 512
    KB = K // 4          # k-tiles bundled per DMA
    nc = bacc.Bacc("TRN2", target_bir_lowering=False, debug=False)
    # CT/WO are host-pretiled so every DMA reads long contiguous
    # per-partition runs (8KB / 4KB) — strided reads from the natural
    # [D, x] layout only sustain ~40% of DMA bandwidth and gate the lead-in.
    # DMA discipline: dma_start costs ~0.6us serial config time on the
    # issuing engine, so transfers are few and large.  CT + XIN ride the
    # SP (sync) queue, WO rides the Activation queue (its prelude configs
    # precede all Act compute in program order).
    CT = nc.dram_tensor("CT", [KB, P, 4, TPC], BF, kind="ExternalInput")
    XIN = nc.dram_tensor("XIN", [TPC, D], BF, kind="ExternalInput")
    WO = nc.dram_tensor("WO", [NC4, KB, P, 4, 512], BF, kind="ExternalInput")
    NW = nc.dram_tensor("NW", [D], F32, kind="ExternalInput")
    Y = nc.dram_tensor("Y", [TPC, D], BF, kind="ExternalOutput")

    XINr = XIN[:, :].rearrange("(m p) d -> p m d", p=P)

    with tile.TileContext(nc) as tc:
        with (
            tc.tile_pool(name="consts", bufs=1) as consts,
            tc.tile_pool(name="ct", bufs=1) as ctp,
            tc.tile_pool(name="wo", bufs=3) as wop,
            tc.tile_pool(name="yall", bufs=1) as yallp,
            tc.tile_pool(name="sq", bufs=3) as sqp,
            tc.tile_pool(name="xin", bufs=2) as xinp,
            tc.tile_pool(name="yn", bufs=2) as ynp,
            tc.tile_pool(name="ssm", bufs=1) as ssmp,
            tc.tile_pool(name="stat", bufs=4) as statp,
            tc.tile_pool(name="ps", bufs=1, space="PSUM") as psp,
        ):
            # CT as one SBUF-resident tile; first bundle k-split so the
            # n=0 k-loop starts after ~0.4MB.
            ct_all = ctp.tile([P, KB, 4, TPC], BF)
            # first k-tile split by m-range: the k=0/m=0 matmul only gates
            # on 64KB of CT + 128KB of WO.  CT's last bundle rides the
            # scalar queue (between wo-n0 and wo-n1) -- the n=0 k-loop
            # consumes 6MB in 17us, which needs both queues flat out.
            nc.sync.dma_start(ct_all[:, 0, 0:1, 0:256], CT[0, :, 0:1, 0:256])
            wo_cur = wop.tile([P, KB, 4, 512], BF, tag="wo", name="wo")
            WOr = [WO[n].rearrange("b p j x -> p b j x") for n in range(NC4)]
            nc.scalar.dma_start(wo_cur[:, 0, 0:1], WOr[0][:, 0, 0:1])
            nc.sync.dma_start(ct_all[:, 0, 0:1, 256:TPC], CT[0, :, 0:1, 256:TPC])
            nc.scalar.dma_start(wo_cur[:, 0, 1:4], WOr[0][:, 0, 1:4])
            nc.sync.dma_start(ct_all[:, 0, 1:4], CT[0, :, 1:4])
            nc.scalar.dma_start(wo_cur[:, 1:KB], WOr[0][:, 1:KB])
            nc.sync.dma_start(ct_all[:, 1], CT[1])
            nc.sync.dma_start(ct_all[:, 2], CT[2])
            nc.scalar.dma_start(ct_all[:, 3], CT[3])
            wo_nxt = wop.tile([P, KB, 4, 512], BF, tag="wo", name="wo")
            nc.scalar.dma_start(wo_nxt[:, 0:1], WOr[1][:, 0:1])
            nc.scalar.dma_start(wo_nxt[:, 1:KB], WOr[1][:, 1:KB])
            # y_all accumulator (fp32); the residual XIN streams in as
            # per-(m,n) bf16 slices added at psum-eviction time, so its
            # bytes never compete with the lead-in wo/ct stream.
            y_all = yallp.tile([P, M, D], F32)
            nw_sb = None
            if not unit_nw:
                nw_sb = consts.tile([P, D], F32)
                nwap = NW[:]
                nw_bcast = bass.AP(
                    tensor=nwap.tensor, offset=nwap.offset, ap=[[0, P]] + list(nwap.ap)
                )
                nc.sync.dma_start(nw_sb[:], nw_bcast)
            eps_sb = consts.tile([P, 1], F32)
            nc.vector.memset(eps_sb[:], eps)

            ssm_t = ssmp.tile([P, M], F32)
            ss_m = [ssm_t[:, m : m + 1] for m in range(M)]

            def stats(m, n, y_slice, first):
                # incremental RMS stats: ss_m[m] += sum(y_slice^2)
                ncols = y_slice.shape[-1]
                sq = sqp.tile([P, 1024], F32, tag="sq", name="sq")
                ssp = statp.tile([P, 1], F32, tag="ssp", name="ssp")
                nc.scalar.activation(
                    sq[:, :ncols],
                    y_slice,
                    mybir.ActivationFunctionType.Square,
                    accum_out=ssp[:],
                )
                if first:
                    nc.vector.tensor_copy(ss_m[m], ssp[:])
                else:
                    nc.vector.tensor_add(ss_m[m], ss_m[m], ssp[:])

            def epilogue(m):
                y_m = y_all[:, m, :]
                rms = statp.tile([P, 1], F32, tag="rms", name="rms")
                nc.scalar.activation(
                    rms[:],
                    ss_m[m],
                    mybir.ActivationFunctionType.Sqrt,
                    bias=eps_sb[:],
                    scale=1.0 / D,
                )
                rstd = statp.tile([P, 1], F32, tag="rstd", name="rstd")
                nc.vector.reciprocal(rstd[:], rms[:])
                yn = ynp.tile([P, D], BF, tag="yn", name="yn")
                if unit_nw and m == M - 1:
                    # final m: its scale trails the last matmul; pipeline
                    # 512-col pieces across scalar+vector and both HWDGE
                    # queues so the last Y write starts ASAP
                    for q in range(4):
                        qs = slice(q * 512, (q + 1) * 512)
                        if q % 2 == 0:
                            nc.scalar.activation(
                                yn[:, qs],
                                y_all[:, m, qs],
                                mybir.ActivationFunctionType.Identity,
                                bias=0.0,
                                scale=rstd[:],
                            )
                        else:
                            nc.vector.tensor_scalar_mul(
                                yn[:, qs], y_all[:, m, qs], rstd[:]
                            )
                        eng = nc.sync if q % 2 == 0 else nc.scalar
                        eng.dma_start(Y[m * P : (m + 1) * P, qs], yn[:, qs])
                    return
                for h in range(1):
                    hs = slice(0, D)
                    if unit_nw:
                        # split each scale across scalar+vector so neither
                        # engine's per-m epilogue load exceeds the 3.4us/m
                        # tensor pace and the trailing chain stays short
                        nc.scalar.activation(
                            yn[:, 0 : D // 2],
                            y_all[:, m, 0 : D // 2],
                            mybir.ActivationFunctionType.Identity,
                            bias=0.0,
                            scale=rstd[:],
                        )
                        nc.vector.tensor_scalar_mul(
                            yn[:, D // 2 : D], y_all[:, m, D // 2 : D], rstd[:]
                        )
                    else:
                        nc.vector.scalar_tensor_tensor(
                            yn[:, hs],
                            y_all[:, m, hs],
                            rstd[:],
                            nw_sb[:, hs],
                            op0=mybir.AluOpType.mult,
                            op1=mybir.AluOpType.mult,
                        )
                    nc.sync.dma_start(Y[m * P : (m + 1) * P, hs], yn[:, hs])

            pss = [psp.tile([P, 512], F32, tag=f"ps{m}", name=f"ps{m}")
                   for m in range(M)]

            # PE warm-up during the DMA lead-in (see build_l1)
            warm = consts.tile([P, 64], BF)
            nc.vector.memset(warm[:], 1.0)
            for _ in range(48):
                nc.tensor.matmul(pss[0][0:64, 0:64], lhsT=warm[:, 0:64],
                                 rhs=warm[:], start=True, stop=True)

            xin_prev = None
            for n in range(NC4):
                n0 = n * 512
                # one batched XIN load per n-chunk
                xin_n = xinp.tile([P, M, 512], BF, tag="xin", name="xin")
                nc.sync.dma_start(xin_n[:], XINr[:, :, n0 : n0 + 512])
                if n + 2 < NC4:
                    wo_n2 = wop.tile([P, KB, 4, 512], BF, tag="wo", name="wo")
                    nc.scalar.dma_start(wo_n2[:], WOr[n + 2])
                if n == 0:
                    # k-outer for the first chunk: consumes each (wo, ct)
                    # bundle over 8 matmuls, pacing the k-loop to the DMA
                    # stream instead of stalling m=0 on the full 6MB.
                    for k in range(K):
                        for m in range(M):
                            nc.tensor.matmul(
                                pss[m][:],
                                lhsT=ct_all[:, k // 4, k % 4, m * P : (m + 1) * P],
                                rhs=wo_cur[:, k // 4, k % 4],
                                start=(k == 0),
                                stop=(k == K - 1),
                            )
                    for m in range(M):
                        # defer the n=0 residual add: evict psum with a
                        # plain copy so nothing here waits on XIN
                        nc.vector.tensor_copy(y_all[:, m, 0:512], pss[m][:])
                else:
                    # m-outer, k-inner: 16 consecutive matmuls accumulate
                    # into one PSUM bank before it's read (avoids psum-queue
                    # depth-cycling micro-idles).
                    for m in range(M):
                        for k in range(K):
                            nc.tensor.matmul(
                                pss[m][:],
                                lhsT=ct_all[:, k // 4, k % 4, m * P : (m + 1) * P],
                                rhs=wo_cur[:, k // 4, k % 4],
                                start=(k == 0),
                                stop=(k == K - 1),
                            )
                        y_slice = y_all[:, m, n0 : n0 + 512]
                        nc.vector.tensor_add(y_slice, xin_n[:, m], pss[m][:])
                        if n == 1:
                            # catch up n=0's deferred residual add and run
                            # stats over both chunks at once
                            nc.vector.tensor_add(
                                y_all[:, m, 0:512], y_all[:, m, 0:512],
                                xin_prev[:, m],
                            )
                            stats(m, n, y_all[:, m, 0:1024], first=True)
                        else:
                            stats(m, n, y_slice, first=False)
                        if n + 1 == NC4:
                            # chain each m's epilogue behind its own k-loop
                            # so only m=M-1's trails the final matmul
                            epilogue(m)
                xin_prev = xin_n
                if n + 1 < NC4:
                    wo_cur = wo_nxt
                    if n + 2 < NC4:
                        wo_nxt = wo_n2
    nc.compile()
    return nc


def host_dispatch(xf, Wr, br):
    """Router + top-2 + softmax gates + expert grouping. Returns dispatch info."""
    T, D = xf.shape
    E = Wr.shape[1]
    logits = xf @ Wr + br
    i1 = np.argmax(logits, axis=1)
    l2 = logits.copy()
    l2[np.arange(T), i1] = -np.inf
    i2 = np.argmax(l2, axis=1)
    v1 = logits[np.arange(T), i1]
    v2 = logits[np.arange(T), i2]
    e2 = np.exp(v2 - v1)
    g1 = (1.0 / (1.0 + e2)).astype(np.float32)
    g2 = (e2 / (1.0 + e2)).astype(np.float32)

    # flat slots (t,s) grouped by expert, stable by (token, slot)
    ee = np.stack([i1, i2], 1).ravel()          # [2T]
    gg = np.stack([g1, g2], 1).ravel()
    tt = np.repeat(np.arange(T), 2)
    order = np.argsort(ee, kind="stable")
    counts = np.bincount(ee, minlength=E)
    starts = np.concatenate([[0], np.cumsum(counts)[:-1]])
    rank = np.empty(2 * T, np.int64)
    rank[order] = np.arange(2 * T)
    pos = rank - starts[ee]                      # position within expert's list
    return dict(
        e1=i1, e2=i2, counts=counts, order=order, starts=starts,
        pos=pos.reshape(T, 2), tok=tt, gate=gg, g1=g1, g2=g2,
    )


def prep_l1_inputs(xf, d, We, be):
    """Per-expert L1 inputs: gathered+pretiled fp8 XT, fp8 W (x WSCALE),
    fp32 be.  Tokens past CAP (expert overflow) are evaluated here on host
    in fp32 and stashed in d["Hov"] for the combine.
    """
    T, D = xf.shape
    E = We.shape[0]
    K2 = D // P // 2
    F = D // P
    C = CAP // 512
    counts = d["counts"]
    We_f = np.asarray(We, np.float32)
    be_f = np.asarray(be, np.float32)
    in1 = []
    Hov = []
    for e in range(E):
        n_e = int(counts[e])
        sel = d["order"][d["starts"][e] : d["starts"][e] + n_e]
        toks = d["tok"][sel]
        n_dev = min(n_e, CAP)
        Xg = np.zeros((CAP, D), np.float32)
        Xg[:n_dev] = xf[toks[:n_dev]]
        # [C, P, K2, 2, n]: DoubleRow k-pair groups, contiguous 8KB
        # per-partition DMA runs
        XT_T = np.ascontiguousarray(
            Xg.astype(F8NP).reshape(C, 512, K2, 2, P).transpose(0, 4, 2, 3, 1)
        )
        # [P, F, K2, 2, P]: partition-major so f-tile slabs DMA as single
        # transfers with (4*f_span)KB per-partition runs
        W_T = np.ascontiguousarray(
            (We_f[e] * WSCALE).astype(F8NP)
            .reshape(K2, 2, P, F, P).transpose(2, 3, 0, 1, 4)
        )
        in1.append({"XT": XT_T, "W": W_T, "BE": be_f[e]})
        if n_e > CAP:
            Xov = xf[toks[CAP:]]                        # [m, D] fp32
            pre = Xov @ We_f[e] + be_f[e]
            ho = (pre / (1.0 + np.exp(-pre))).astype(np.float32)
            Hov.append(ho.T)                            # [D, m]
        else:
            Hov.append(np.zeros((D, 0), np.float32))
    d["Hov"] = Hov
    return in1, CAP


def prep_l2_inputs(xf, d, H, Wo, bo, norm_w):
    """Per-core L2 inputs. CT = g1*A + g2*B combined on host (fp32 math,
    one bf16 downcast); XIN = x + bo in fp32."""
    T, D = xf.shape
    TPC = T // NCORE
    KB = D // P // 4
    NC4 = D // 512
    # pretile Wo into contiguous (n-chunk, k-bundle) blocks
    Wo_b = np.ascontiguousarray(
        np.asarray(Wo, np.float32)
        .reshape(KB, 4, P, NC4, 512)
        .transpose(3, 0, 2, 1, 4)
    ).astype(BF16)
    bo_f = np.asarray(bo, np.float32)
    nw_f = np.asarray(norm_w, np.float32)
    e1, e2, pos = d["e1"], d["e2"], d["pos"]
    g1, g2 = d["g1"], d["g2"]
    # device H (raw [C, FQ, P, 4, 512] batches -> [D, CAP], first CAP
    # slots) + host-computed overflow columns
    def unpack_h(h_raw):
        return np.ascontiguousarray(
            np.asarray(h_raw, np.float32).transpose(1, 3, 2, 0, 4).reshape(D, CAP)
        )
    Hfull = [np.concatenate([unpack_h(H[e]), d["Hov"][e]], axis=1)
             for e in range(H.shape[0])]
    in2 = []
    for c in range(NCORE):
        tl = np.arange(c * TPC, (c + 1) * TPC)
        CTf = np.empty((D, TPC), np.float32)
        BTf = np.empty((D, TPC), np.float32)
        for e in range(H.shape[0]):
            s1 = e1[tl] == e
            if s1.any():
                CTf[:, s1] = Hfull[e][:, pos[tl[s1], 0]]
            s2 = e2[tl] == e
            if s2.any():
                BTf[:, s2] = Hfull[e][:, pos[tl[s2], 1]]
        CTf = CTf * g1[tl][None, :] + BTf * g2[tl][None, :]
        CTt = np.ascontiguousarray(
            CTf.reshape(KB, 4, P, TPC).transpose(0, 2, 1, 3)
        ).astype(BF16)
        XIN = (xf[tl] + bo_f[None, :]).astype(BF16)
        in2.append({"CT": CTt, "XIN": XIN, "WO": Wo_b, "NW": nw_f})
    return in2


# ----------------------------------------------------------------------------
# Harness entry point: full (unsharded) inputs -> full output.
# ----------------------------------------------------------------------------
_L1_CACHE = {}
_L2_CACHE = {}


def kernel(x, Wr, br, We, be, Wo, bo, norm_w):
    B, S, D = x.shape
    E = We.shape[0]
    T = B * S
    TPC = T // NCORE
    xf = np.ascontiguousarray(np.asarray(x, np.float32).reshape(T, D))
    d = host_dispatch(xf, np.asarray(Wr, np.float32), np.asarray(br, np.float32))

    in1, Bcap = prep_l1_inputs(xf, d, We, be)
    if (D, Bcap) not in _L1_CACHE:
        _L1_CACHE[(D, Bcap)] = build_l1(D, Bcap)
    r1 = run_bass_kernel_spmd(_L1_CACHE[(D, Bcap)], in1, list(range(NCORE)))
    H = np.stack([r1.results[e]["H"] for e in range(E)])

    in2 = prep_l2_inputs(xf, d, H, Wo, bo, norm_w)
    unit_nw = bool(np.all(np.asarray(norm_w, np.float32) == 1.0))
    if (D, TPC, unit_nw) not in _L2_CACHE:
        _L2_CACHE[(D, TPC, unit_nw)] = build_l2(D, TPC, unit_nw)
    r2 = run_bass_kernel_spmd(_L2_CACHE[(D, TPC, unit_nw)], in2, list(range(NCORE)))
    Y = np.concatenate([r2.results[c]["Y"] for c in range(NCORE)], axis=0)
    return Y.reshape(B, S, D).astype(np.asarray(x).dtype)



# revision 33
# speedup vs baseline: 1.7414x; 1.0283x over previous
# MoE EnhancedGatedFusion kernel for 8x TRN2 NeuronCores (expert-parallel).
#
# Decomposition:
#   host : router logits -> top2 -> softmax gates -> dispatch by expert
#   L1   : per-core (expert e): H_T[d_out, n] = silu(We[e].T-contract @ XT + be[e])
#          (ungated; fp8e4 DoubleRow matmuls, fp32 PSUM, bf16 H output)
#   host : combine CT = g1*A + g2*B during the token un-shuffle (the
#          "all-to-all return" glue), downcast once to bf16; tokens past the
#          per-expert capacity (CAP) are the standard MoE overflow case and
#          are evaluated on host in fp32 during the combine.
#   L2   : per-core (1024 tokens): out = CT.T @ Wo; y = XIN + out (XIN =
#          x_shard + bo folded on host, fp32); RMS-norm * norm_w.
#
# L1 matmuls run fp8e4 (e4m3) in DoubleRow perf mode: each instruction
# contracts 256 rows (two 128-deep k-groups) at 1 output column/cycle --
# measured 1.96x the bf16 MAC rate on hardware.  Weights are prescaled by
# 64 on host to clear the e4m3 subnormal range (std 0.022 -> 1.4) and the
# 1/64 is folded into the silu activation's input scale.  End-to-end rel
# err of the fp8 L1 + bf16 L2 pipeline is ~1.6e-2 (gate: 2e-2).
# L2 stays bf16: fp8 there would push total error past the gate.
import sys
import types

sys.path.insert(0, "/opt/trn_rl_repo")

import numpy as np
import ml_dtypes

BF16 = np.dtype(ml_dtypes.bfloat16)


def _install_ntff_hook():
    # antenv.axon_hooks is missing in this image; shim it so
    # run_bass_kernel_spmd(trace=True) can drive NTFF profiling.
    if "antenv.axon_hooks" in sys.modules:
        return
    try:
        from trn_agent_boot.trn_boot import _ntff_profile_via_ctypes

        hook = _ntff_profile_via_ctypes("/opt/axon/libaxon_pjrt.so")
    except Exception:
        hook = None
    mod = types.ModuleType("antenv.axon_hooks")
    mod.get_axon_ntff_profile_hook = lambda: hook
    mod.set_axon_ntff_profile_hook = lambda h: None
    sys.modules["antenv.axon_hooks"] = mod


_install_ntff_hook()

import concourse.bacc as bacc
import concourse.bass as bass
import concourse.tile as tile
from concourse import mybir
from concourse.bass_utils import run_bass_kernel_spmd

F32 = mybir.dt.float32
BF = mybir.dt.bfloat16
F8 = mybir.dt.float8e4
F8NP = np.dtype(ml_dtypes.float8_e4m3)  # TRN fp8e4: max normal 240
P = 128
NCORE = 8
CAP = 2048      # per-expert token capacity on device; overflow runs on host
WSCALE = 64.0   # We prescale before fp8 cast (undone in the silu's scale)


def _chunks(total, size):
    out = []
    o = 0
    while o < total:
        out.append((o, min(size, total - o)))
        o += size
    return out


def build_l1(D, cap):
    """Per-core expert FFN: H[d_out, n] = silu((1/WSCALE)*sum_k W[k,d_out]*XT[k,n] + be[d_out]).

    fp8e4 DoubleRow matmuls: XT is host-pretiled fp8 [C, P, K2, 2, 512]
    (k-tiles paired into DoubleRow groups) and W is fp8 [P, F, K2, 2, P]
    (partition-major so multi-f-tile slabs DMA as one transfer with long
    per-partition runs), prescaled by WSCALE on host.  W is fully
    SBUF-resident (4.2MB), XT streams through once.  H output is bf16,
    written in 4-f-tile batches [C, FQ, P, 4, 512] so each store DMA has
    4KB per-partition runs.

    DMA discipline: every dma_start costs ~0.6us of serial config time on
    the issuing engine's sequencer, so transfers are few and large.  All
    loads ride the SP (sync) HWDGE queue (no compute there); H stores ride
    the Activation queue, whose configs interleave naturally with the
    silus that produce them.
    """
    K2 = D // P // 2    # DoubleRow k-pairs
    F = D // P          # feat (d_out) tiles
    FQ = F // 4         # H store batches
    C = cap // 512      # 512-col chunks
    assert cap % 512 == 0
    nc = bacc.Bacc("TRN2", target_bir_lowering=False, debug=False)
    XT = nc.dram_tensor("XT", [C, P, K2, 2, 512], F8, kind="ExternalInput")
    W = nc.dram_tensor("W", [P, F, K2, 2, P], F8, kind="ExternalInput")
    BE = nc.dram_tensor("BE", [D], F32, kind="ExternalInput")
    H = nc.dram_tensor("H", [C, FQ, P, 4, 512], BF, kind="ExternalOutput")
    DR = mybir.MatmulPerfMode.DoubleRow

    with tile.TileContext(nc) as tc:
        with (
            tc.tile_pool(name="consts", bufs=1) as consts,
            tc.tile_pool(name="xt", bufs=3) as xtp,
            tc.tile_pool(name="hout", bufs=3) as hp,
            tc.tile_pool(name="ps", bufs=4, space="PSUM") as psp,
        ):
            # PE warm-up: ~48 tiny matmuls during the DMA lead-in keep the
            # HAM activity window busy so real matmuls start at 2.4GHz.
            warm = consts.tile([P, 64], BF)
            nc.vector.memset(warm[:], 1.0)
            wps = psp.tile([P, 512], F32, tag="ps", name="ps")
            for _ in range(48):
                nc.tensor.matmul(wps[0:64, 0:64], lhsT=warm[:, 0:64],
                                 rhs=warm[:], start=True, stop=True)

            # sync-queue load order = consumption order: W0/XT0 k-pieces
            # first so the f=0 k-loop starts ~3us in, then W slabs paced
            # against the 1.8us/f-tile burn rate, XT chunks behind.
            w_all = consts.tile([P, F, K2, 2, P], F8)
            xt_tiles = {}
            xt_tiles[0] = xtp.tile([P, K2, 2, 512], F8, tag="xt", name="xt")
            nc.sync.dma_start(w_all[:, 0, 0:2], W[:, 0, 0:2])
            nc.sync.dma_start(xt_tiles[0][:, 0:2], XT[0, :, 0:2])
            nc.sync.dma_start(w_all[:, 0, 2 : K2], W[:, 0, 2 : K2])
            nc.sync.dma_start(xt_tiles[0][:, 2:4], XT[0, :, 2:4])
            nc.sync.dma_start(xt_tiles[0][:, 4 : K2], XT[0, :, 4 : K2])
            be_sb = consts.tile([P, F], F32)
            nc.sync.dma_start(be_sb[:], BE[:].rearrange("(f p) -> p f", p=P))
            nc.sync.dma_start(w_all[:, 1:3], W[:, 1:3])
            nc.sync.dma_start(w_all[:, 3:7], W[:, 3:7])
            nc.sync.dma_start(w_all[:, 7:11], W[:, 7:11])
            nc.sync.dma_start(w_all[:, 11:F], W[:, 11:F])
            for ci in range(1, C):
                xt_c = xtp.tile([P, K2, 2, 512], F8, tag="xt", name="xt")
                nc.sync.dma_start(xt_c[:], XT[ci])
                xt_tiles[ci] = xt_c

            for ci in range(C):
                xt_c = xt_tiles[ci]
                h4 = None
                for f in range(F):
                    if f % 4 == 0:
                        h4 = hp.tile([P, 4, 512], BF, tag="h4", name="h4")
                    ps = psp.tile([P, 512], F32, tag="ps", name="ps")
                    for k in range(K2):
                        nc.tensor.matmul(
                            ps[:],
                            lhsT=w_all[:, f, k],
                            rhs=xt_c[:, k],
                            start=(k == 0),
                            stop=(k == K2 - 1),
                            perf_mode=DR,
                        )
                    nc.scalar.activation(
                        h4[:, f % 4],
                        ps[:],
                        mybir.ActivationFunctionType.Silu,
                        bias=be_sb[:, f : f + 1],
                        scale=1.0 / WSCALE,
                    )
                    last_batch = ci == C - 1 and f // 4 == FQ - 1
                    if last_batch and f % 4 == 1:
                        # stream the final batch per-silu so the drain after
                        # the last silu is one f-tile on the idle sync queue
                        nc.scalar.dma_start(H[ci, f // 4, :, 0:2], h4[:, 0:2])
                    elif last_batch and f % 4 == 2:
                        nc.scalar.dma_start(H[ci, f // 4, :, 2:3], h4[:, 2:3])
                    elif last_batch and f % 4 == 3:
                        nc.sync.dma_start(H[ci, f // 4, :, 3:4], h4[:, 3:4])
                    elif f % 4 == 3:
                        nc.scalar.dma_start(H[ci, f // 4], h4[:])
    nc.compile()
    return nc


def build_l2(D, TPC, unit_nw, eps=1e-6):
    """Per-core combine + output proj + residual + RMS norm.

    Y[t, j] = nw[j] * (XIN[t,j] + sum_k CT[k,t]*Wo[k,j]) / rms(t)
    CT = g1*A + g2*B (host-combined, bf16); XIN = x_shard + bo (fp32).
    Y output is bf16 (host upcasts).

    The last n-chunk runs m-outer so each m's epilogue (RMS + scale +
    store) chains behind its own k-loop and overlaps the next m's
    matmuls; only m=M-1's epilogue trails the final matmul.

    unit_nw=True specializes norm_w == 1 (scale-by-rstd runs as a scalar
    engine activation; multiplying by 1 is exact) so the vector engine
    stays under the per-m tensor budget during the epilogue phase.
    """
    K = D // P
    M = TPC // P
    NC4 = D // 512
    KB = K // 4          # k-tiles bundled per DMA
    KB8 = 1              # leading k-bundles in fp8 (4 k-tiles = 2 DR pairs)
    KBF = KB - KB8       # remaining bf16 k-bundles
    KF = (K - 4 * KB8)   # bf16 k-tiles
    nc = bacc.Bacc("TRN2", target_bir_lowering=False, debug=False)
    # CT/WO are host-pretiled so every DMA reads long contiguous
    # per-partition runs — strided reads from the natural [D, x] layout
    # only sustain ~40% of DMA bandwidth and gate the lead-in.
    # The first 4 k-tiles run as fp8e4 DoubleRow matmuls (CT8/WO8),
    # accumulating into the same PSUM group as the bf16 remainder; all Wo
    # slices are prescaled x64 on host (exact in bf16) and the 1/64 is
    # folded into the psum-eviction op.  This shaves ~12% off L2's PE time
    # for ~0.2e-2 of extra rel err.
    # DMA discipline: dma_start costs ~0.6us serial config time on the
    # issuing engine, so transfers are few and large.  CT + XIN ride the
    # SP (sync) queue, WO rides the Activation queue (its prelude configs
    # precede all Act compute in program order).
    CT8 = nc.dram_tensor("CT8", [P, 2, 2, TPC], F8, kind="ExternalInput")
    CT = nc.dram_tensor("CT", [KBF, P, 4, TPC], BF, kind="ExternalInput")
    XIN = nc.dram_tensor("XIN", [TPC, D], BF, kind="ExternalInput")
    WO8 = nc.dram_tensor("WO8", [NC4, P, 2, 2, 512], F8, kind="ExternalInput")
    WO = nc.dram_tensor("WO", [NC4, KBF, P, 4, 512], BF, kind="ExternalInput")
    NW = nc.dram_tensor("NW", [D], F32, kind="ExternalInput")
    Y = nc.dram_tensor("Y", [TPC, D], BF, kind="ExternalOutput")

    XINr = XIN[:, :].rearrange("(m p) d -> p m d", p=P)
    DR = mybir.MatmulPerfMode.DoubleRow
    EV = 1.0 / WSCALE    # psum carries x64 products

    with tile.TileContext(nc) as tc:
        with (
            tc.tile_pool(name="consts", bufs=1) as consts,
            tc.tile_pool(name="ct", bufs=1) as ctp,
            tc.tile_pool(name="wo", bufs=3) as wop,
            tc.tile_pool(name="yall", bufs=1) as yallp,
            tc.tile_pool(name="sq", bufs=3) as sqp,
            tc.tile_pool(name="xin", bufs=2) as xinp,
            tc.tile_pool(name="yn", bufs=2) as ynp,
            tc.tile_pool(name="ssm", bufs=1) as ssmp,
            tc.tile_pool(name="stat", bufs=4) as statp,
            tc.tile_pool(name="ps", bufs=1, space="PSUM") as psp,
        ):
            # CT as one SBUF-resident tile; first bundle k-split so the
            # n=0 k-loop starts after ~0.4MB.
            # fp8 head is small (0.5MB CT8 + 0.25MB/chunk WO8): the first
            # matmuls gate on it; bf16 CT bundles stream behind on sync,
            # CT's last bundle rides the scalar queue between wo-n0 and
            # wo-n1 so both queues run flat out through the n=0 k-loop.
            ct8_all = ctp.tile([P, 2, 2, TPC], F8, tag="ct8", name="ct8")
            ct_all = ctp.tile([P, KBF, 4, TPC], BF, tag="ctbf", name="ctbf")
            WOr = [WO[n].rearrange("b p j x -> p b j x") for n in range(NC4)]
            wo8_cur = wop.tile([P, 2, 2, 512], F8, tag="wo8", name="wo8")
            wo_cur = wop.tile([P, KBF, 4, 512], BF, tag="wo", name="wo")
            nc.sync.dma_start(ct8_all[:, :, :, 0:256], CT8[:, :, :, 0:256])
            nc.scalar.dma_start(wo8_cur[:], WO8[0])
            nc.sync.dma_start(ct8_all[:, :, :, 256:TPC], CT8[:, :, :, 256:TPC])
            nc.scalar.dma_start(wo_cur[:], WOr[0])
            nc.sync.dma_start(ct_all[:, 0, 0:2], CT[0, :, 0:2])
            nc.sync.dma_start(ct_all[:, 0, 2:4], CT[0, :, 2:4])
            nc.sync.dma_start(ct_all[:, 1], CT[1])
            nc.scalar.dma_start(ct_all[:, 2], CT[2])
            wo8_nxt = wop.tile([P, 2, 2, 512], F8, tag="wo8", name="wo8")
            wo_nxt = wop.tile([P, KBF, 4, 512], BF, tag="wo", name="wo")
            nc.scalar.dma_start(wo8_nxt[:], WO8[1])
            nc.scalar.dma_start(wo_nxt[:], WOr[1])
            # y_all accumulator (fp32); the residual XIN streams in as
            # per-(m,n) bf16 slices added at psum-eviction time, so its
            # bytes never compete with the lead-in wo/ct stream.
            y_all = yallp.tile([P, M, D], F32)
            nw_sb = None
            if not unit_nw:
                nw_sb = consts.tile([P, D], F32)
                nwap = NW[:]
                nw_bcast = bass.AP(
                    tensor=nwap.tensor, offset=nwap.offset, ap=[[0, P]] + list(nwap.ap)
                )
                nc.sync.dma_start(nw_sb[:], nw_bcast)
            eps_sb = consts.tile([P, 1], F32)
            nc.vector.memset(eps_sb[:], eps)

            ssm_t = ssmp.tile([P, M], F32)
            ss_m = [ssm_t[:, m : m + 1] for m in range(M)]

            def stats(m, n, y_slice, first):
                # incremental RMS stats: ss_m[m] += sum(y_slice^2)
                ncols = y_slice.shape[-1]
                sq = sqp.tile([P, 1024], F32, tag="sq", name="sq")
                ssp = statp.tile([P, 1], F32, tag="ssp", name="ssp")
                nc.scalar.activation(
                    sq[:, :ncols],
                    y_slice,
                    mybir.ActivationFunctionType.Square,
                    accum_out=ssp[:],
                )
                if first:
                    nc.vector.tensor_copy(ss_m[m], ssp[:])
                else:
                    nc.vector.tensor_add(ss_m[m], ss_m[m], ssp[:])

            def epilogue(m):
                y_m = y_all[:, m, :]
                rms = statp.tile([P, 1], F32, tag="rms", name="rms")
                nc.scalar.activation(
                    rms[:],
                    ss_m[m],
                    mybir.ActivationFunctionType.Sqrt,
                    bias=eps_sb[:],
                    scale=1.0 / D,
                )
                rstd = statp.tile([P, 1], F32, tag="rstd", name="rstd")
                nc.vector.reciprocal(rstd[:], rms[:])
                yn = ynp.tile([P, D], BF, tag="yn", name="yn")
                if unit_nw and m == M - 1:
                    # final m: its scale trails the last matmul; pipeline
                    # 512-col pieces across scalar+vector and both HWDGE
                    # queues so the last Y write starts ASAP
                    for q in range(4):
                        qs = slice(q * 512, (q + 1) * 512)
                        if q % 2 == 0:
                            nc.scalar.activation(
                                yn[:, qs],
                                y_all[:, m, qs],
                                mybir.ActivationFunctionType.Identity,
                                bias=0.0,
                                scale=rstd[:],
                            )
                        else:
                            nc.vector.tensor_scalar_mul(
                                yn[:, qs], y_all[:, m, qs], rstd[:]
                            )
                        eng = nc.sync if q % 2 == 0 else nc.scalar
                        eng.dma_start(Y[m * P : (m + 1) * P, qs], yn[:, qs])
                    return
                for h in range(1):
                    hs = slice(0, D)
                    if unit_nw:
                        # split each scale across scalar+vector so neither
                        # engine's per-m epilogue load exceeds the 3.4us/m
                        # tensor pace and the trailing chain stays short
                        nc.scalar.activation(
                            yn[:, 0 : D // 2],
                            y_all[:, m, 0 : D // 2],
                            mybir.ActivationFunctionType.Identity,
                            bias=0.0,
                            scale=rstd[:],
                        )
                        nc.vector.tensor_scalar_mul(
                            yn[:, D // 2 : D], y_all[:, m, D // 2 : D], rstd[:]
                        )
                    else:
                        nc.vector.scalar_tensor_tensor(
                            yn[:, hs],
                            y_all[:, m, hs],
                            rstd[:],
                            nw_sb[:, hs],
                            op0=mybir.AluOpType.mult,
                            op1=mybir.AluOpType.mult,
                        )
                    nc.sync.dma_start(Y[m * P : (m + 1) * P, hs], yn[:, hs])

            pss = [psp.tile([P, 512], F32, tag=f"ps{m}", name=f"ps{m}")
                   for m in range(M)]

            # PE warm-up during the DMA lead-in (see build_l1)
            warm = consts.tile([P, 64], BF)
            nc.vector.memset(warm[:], 1.0)
            for _ in range(48):
                nc.tensor.matmul(pss[0][0:64, 0:64], lhsT=warm[:, 0:64],
                                 rhs=warm[:], start=True, stop=True)

            xin_prev = None
            for n in range(NC4):
                n0 = n * 512
                # one batched XIN load per n-chunk
                xin_n = xinp.tile([P, M, 512], BF, tag="xin", name="xin")
                nc.sync.dma_start(xin_n[:], XINr[:, :, n0 : n0 + 512])
                if n + 2 < NC4:
                    wo8_n2 = wop.tile([P, 2, 2, 512], F8, tag="wo8", name="wo8")
                    wo_n2 = wop.tile([P, KBF, 4, 512], BF, tag="wo", name="wo")
                    nc.scalar.dma_start(wo8_n2[:], WO8[n + 2])
                    nc.scalar.dma_start(wo_n2[:], WOr[n + 2])
                def mm_f8(m, kp, start):
                    nc.tensor.matmul(
                        pss[m][:],
                        lhsT=ct8_all[:, kp, :, m * P : (m + 1) * P],
                        rhs=wo8_cur[:, kp],
                        start=start,
                        stop=False,
                        perf_mode=DR,
                    )

                def mm_bf(m, k, stop):
                    nc.tensor.matmul(
                        pss[m][:],
                        lhsT=ct_all[:, k // 4, k % 4, m * P : (m + 1) * P],
                        rhs=wo_cur[:, k // 4, k % 4],
                        start=False,
                        stop=stop,
                    )

                if n == 0:
                    # k-outer for the first chunk: consumes each (wo, ct)
                    # bundle over 8 matmuls, pacing the k-loop to the DMA
                    # stream instead of stalling m=0 on the full 6MB.
                    for kp in range(2):
                        for m in range(M):
                            mm_f8(m, kp, kp == 0)
                    for k in range(KF):
                        for m in range(M):
                            mm_bf(m, k, k == KF - 1)
                    for m in range(M):
                        # defer the n=0 residual add: evict psum (x64) with
                        # a scaled copy so nothing here waits on XIN
                        nc.vector.tensor_scalar_mul(y_all[:, m, 0:512],
                                                    pss[m][:], EV)
                else:
                    # m-outer, k-inner: 16 consecutive matmuls accumulate
                    # into one PSUM bank before it's read (avoids psum-queue
                    # depth-cycling micro-idles).
                    for m in range(M):
                        for kp in range(2):
                            mm_f8(m, kp, kp == 0)
                        for k in range(KF):
                            mm_bf(m, k, k == KF - 1)
                        y_slice = y_all[:, m, n0 : n0 + 512]
                        # y = psum/64 + xin
                        nc.vector.scalar_tensor_tensor(
                            y_slice, pss[m][:], EV, xin_n[:, m],
                            op0=mybir.AluOpType.mult,
                            op1=mybir.AluOpType.add,
                        )
                        if n == 1:
                            # catch up n=0's deferred residual add and run
                            # stats over both chunks at once
                            nc.vector.tensor_add(
                                y_all[:, m, 0:512], y_all[:, m, 0:512],
                                xin_prev[:, m],
                            )
                            stats(m, n, y_all[:, m, 0:1024], first=True)
                        else:
                            stats(m, n, y_slice, first=False)
                        if n + 1 == NC4:
                            # chain each m's epilogue behind its own k-loop
                            # so only m=M-1's trails the final matmul
                            epilogue(m)
                xin_prev = xin_n
                if n + 1 < NC4:
                    wo_cur = wo_nxt
                    wo8_cur = wo8_nxt
                    if n + 2 < NC4:
                        wo_nxt = wo_n2
                        wo8_nxt = wo8_n2
    nc.compile()
    return nc


def host_dispatch(xf, Wr, br):
    """Router + top-2 + softmax gates + expert grouping. Returns dispatch info."""
    T, D = xf.shape
    E = Wr.shape[1]
    logits = xf @ Wr + br
    i1 = np.argmax(logits, axis=1)
    l2 = logits.copy()
    l2[np.arange(T), i1] = -np.inf
    i2 = np.argmax(l2, axis=1)
    v1 = logits[np.arange(T), i1]
    v2 = logits[np.arange(T), i2]
    e2 = np.exp(v2 - v1)
    g1 = (1.0 / (1.0 + e2)).astype(np.float32)
    g2 = (e2 / (1.0 + e2)).astype(np.float32)

    # flat slots (t,s) grouped by expert, stable by (token, slot)
    ee = np.stack([i1, i2], 1).ravel()          # [2T]
    gg = np.stack([g1, g2], 1).ravel()
    tt = np.repeat(np.arange(T), 2)
    order = np.argsort(ee, kind="stable")
    counts = np.bincount(ee, minlength=E)
    starts = np.concatenate([[0], np.cumsum(counts)[:-1]])
    rank = np.empty(2 * T, np.int64)
    rank[order] = np.arange(2 * T)
    pos = rank - starts[ee]                      # position within expert's list
    return dict(
        e1=i1, e2=i2, counts=counts, order=order, starts=starts,
        pos=pos.reshape(T, 2), tok=tt, gate=gg, g1=g1, g2=g2,
    )


def prep_l1_inputs(xf, d, We, be):
    """Per-expert L1 inputs: gathered+pretiled fp8 XT, fp8 W (x WSCALE),
    fp32 be.  Tokens past CAP (expert overflow) are evaluated here on host
    in fp32 and stashed in d["Hov"] for the combine.
    """
    T, D = xf.shape
    E = We.shape[0]
    K2 = D // P // 2
    F = D // P
    C = CAP // 512
    counts = d["counts"]
    We_f = np.asarray(We, np.float32)
    be_f = np.asarray(be, np.float32)
    in1 = []
    Hov = []
    for e in range(E):
        n_e = int(counts[e])
        sel = d["order"][d["starts"][e] : d["starts"][e] + n_e]
        toks = d["tok"][sel]
        n_dev = min(n_e, CAP)
        Xg = np.zeros((CAP, D), np.float32)
        Xg[:n_dev] = xf[toks[:n_dev]]
        # [C, P, K2, 2, n]: DoubleRow k-pair groups, contiguous 8KB
        # per-partition DMA runs
        XT_T = np.ascontiguousarray(
            Xg.astype(F8NP).reshape(C, 512, K2, 2, P).transpose(0, 4, 2, 3, 1)
        )
        # [P, F, K2, 2, P]: partition-major so f-tile slabs DMA as single
        # transfers with (4*f_span)KB per-partition runs
        W_T = np.ascontiguousarray(
            (We_f[e] * WSCALE).astype(F8NP)
            .reshape(K2, 2, P, F, P).transpose(2, 3, 0, 1, 4)
        )
        in1.append({"XT": XT_T, "W": W_T, "BE": be_f[e]})
        if n_e > CAP:
            Xov = xf[toks[CAP:]]                        # [m, D] fp32
            pre = Xov @ We_f[e] + be_f[e]
            ho = (pre / (1.0 + np.exp(-pre))).astype(np.float32)
            Hov.append(ho.T)                            # [D, m]
        else:
            Hov.append(np.zeros((D, 0), np.float32))
    d["Hov"] = Hov
    return in1, CAP


def prep_l2_inputs(xf, d, H, Wo, bo, norm_w):
    """Per-core L2 inputs. CT = g1*A + g2*B combined on host (fp32 math);
    the first 512 k-rows go out as fp8 (DoubleRow pairs), the rest bf16.
    All Wo slices are prescaled x64 (exact in bf16; lifts the fp8 rows out
    of e4m3's subnormal range).  XIN = x + bo in fp32."""
    T, D = xf.shape
    TPC = T // NCORE
    KB = D // P // 4
    KBF = KB - 1
    NC4 = D // 512
    Wo_f = np.asarray(Wo, np.float32) * WSCALE
    # fp8 head: [NC4, P, 2, 2, 512] DoubleRow k-pair groups
    Wo8_b = np.ascontiguousarray(
        Wo_f[:512].astype(F8NP)
        .reshape(2, 2, P, NC4, 512).transpose(3, 2, 0, 1, 4)
    )
    # bf16 tail: [NC4, KBF, P, 4, 512]
    Wo_b = np.ascontiguousarray(
        Wo_f[512:].astype(BF16)
        .reshape(KBF, 4, P, NC4, 512).transpose(3, 0, 2, 1, 4)
    )
    bo_f = np.asarray(bo, np.float32)
    nw_f = np.asarray(norm_w, np.float32)
    e1, e2, pos = d["e1"], d["e2"], d["pos"]
    g1, g2 = d["g1"], d["g2"]
    # device H (raw [C, FQ, P, 4, 512] batches -> [D, CAP], first CAP
    # slots) + host-computed overflow columns
    def unpack_h(h_raw):
        return np.ascontiguousarray(
            np.asarray(h_raw, np.float32).transpose(1, 3, 2, 0, 4).reshape(D, CAP)
        )
    Hfull = [np.concatenate([unpack_h(H[e]), d["Hov"][e]], axis=1)
             for e in range(H.shape[0])]
    in2 = []
    for c in range(NCORE):
        tl = np.arange(c * TPC, (c + 1) * TPC)
        CTf = np.empty((D, TPC), np.float32)
        BTf = np.empty((D, TPC), np.float32)
        for e in range(H.shape[0]):
            s1 = e1[tl] == e
            if s1.any():
                CTf[:, s1] = Hfull[e][:, pos[tl[s1], 0]]
            s2 = e2[tl] == e
            if s2.any():
                BTf[:, s2] = Hfull[e][:, pos[tl[s2], 1]]
        CTf = CTf * g1[tl][None, :] + BTf * g2[tl][None, :]
        CT8t = np.ascontiguousarray(
            CTf[:512].astype(F8NP).reshape(2, 2, P, TPC).transpose(2, 0, 1, 3)
        )
        CTt = np.ascontiguousarray(
            CTf[512:].reshape(KBF, 4, P, TPC).transpose(0, 2, 1, 3)
        ).astype(BF16)
        XIN = (xf[tl] + bo_f[None, :]).astype(BF16)
        in2.append({"CT8": CT8t, "CT": CTt, "XIN": XIN,
                    "WO8": Wo8_b, "WO": Wo_b, "NW": nw_f})
    return in2


# ----------------------------------------------------------------------------
# Harness entry point: full (unsharded) inputs -> full output.
# ----------------------------------------------------------------------------
_L1_CACHE = {}
_L2_CACHE = {}


def kernel(x, Wr, br, We, be, Wo, bo, norm_w):
    B, S, D = x.shape
    E = We.shape[0]
    T = B * S
    TPC = T // NCORE
    xf = np.ascontiguousarray(np.asarray(x, np.float32).reshape(T, D))
    d = host_dispatch(xf, np.asarray(Wr, np.float32), np.asarray(br, np.float32))

    in1, Bcap = prep_l1_inputs(xf, d, We, be)
    if (D, Bcap) not in _L1_CACHE:
        _L1_CACHE[(D, Bcap)] = build_l1(D, Bcap)
    r1 = run_bass_kernel_spmd(_L1_CACHE[(D, Bcap)], in1, list(range(NCORE)))
    H = np.stack([r1.results[e]["H"] for e in range(E)])

    in2 = prep_l2_inputs(xf, d, H, Wo, bo, norm_w)
    unit_nw = bool(np.all(np.asarray(norm_w, np.float32) == 1.0))
    if (D, TPC, unit_nw) not in _L2_CACHE:
        _L2_CACHE[(D, TPC, unit_nw)] = build_l2(D, TPC, unit_nw)
    r2 = run_bass_kernel_spmd(_L2_CACHE[(D, TPC, unit_nw)], in2, list(range(NCORE)))
    Y = np.concatenate([r2.results[c]["Y"] for c in range(NCORE)], axis=0)
    return Y.reshape(B, S, D).astype(np.asarray(x).dtype)



# revision 37
# speedup vs baseline: 1.7699x; 1.0163x over previous
# MoE EnhancedGatedFusion kernel for 8x TRN2 NeuronCores (expert-parallel).
#
# Decomposition:
#   host : router logits -> top2 -> softmax gates -> dispatch by expert
#   L1   : per-core (expert e): H_T[d_out, n] = silu(We[e].T-contract @ XT + be[e])
#          (ungated; fp8e4 DoubleRow matmuls, fp32 PSUM, bf16 H output)
#   host : combine CT = g1*A + g2*B during the token un-shuffle (the
#          "all-to-all return" glue), downcast once to bf16; tokens past the
#          per-expert capacity (CAP) are the standard MoE overflow case and
#          are evaluated on host in fp32 during the combine.
#   L2   : per-core (1024 tokens): out = CT.T @ Wo; y = XIN + out (XIN =
#          x_shard + bo folded on host, fp32); RMS-norm * norm_w.
#
# L1 matmuls run fp8e4 (e4m3) in DoubleRow perf mode: each instruction
# contracts 256 rows (two 128-deep k-groups) at 1 output column/cycle --
# measured 1.96x the bf16 MAC rate on hardware.  Weights are prescaled by
# 64 on host to clear the e4m3 subnormal range (std 0.022 -> 1.4) and the
# 1/64 is folded into the silu activation's input scale.  End-to-end rel
# err of the fp8 L1 + bf16 L2 pipeline is ~1.6e-2 (gate: 2e-2).
# L2 stays bf16: fp8 there would push total error past the gate.
import sys
import types

sys.path.insert(0, "/opt/trn_rl_repo")

import numpy as np
import ml_dtypes

BF16 = np.dtype(ml_dtypes.bfloat16)


def _install_ntff_hook():
    # antenv.axon_hooks is missing in this image; shim it so
    # run_bass_kernel_spmd(trace=True) can drive NTFF profiling.
    if "antenv.axon_hooks" in sys.modules:
        return
    try:
        from trn_agent_boot.trn_boot import _ntff_profile_via_ctypes

        hook = _ntff_profile_via_ctypes("/opt/axon/libaxon_pjrt.so")
    except Exception:
        hook = None
    mod = types.ModuleType("antenv.axon_hooks")
    mod.get_axon_ntff_profile_hook = lambda: hook
    mod.set_axon_ntff_profile_hook = lambda h: None
    sys.modules["antenv.axon_hooks"] = mod


_install_ntff_hook()

import concourse.bacc as bacc
import concourse.bass as bass
import concourse.tile as tile
from concourse import mybir
from concourse.bass_utils import run_bass_kernel_spmd

F32 = mybir.dt.float32
BF = mybir.dt.bfloat16
F8 = mybir.dt.float8e4
F8NP = np.dtype(ml_dtypes.float8_e4m3)  # TRN fp8e4: max normal 240
P = 128
NCORE = 8
CAP = 2048      # per-expert token capacity on device; overflow runs on host
WSCALE = 64.0   # We prescale before fp8 cast (undone in the silu's scale)


def _chunks(total, size):
    out = []
    o = 0
    while o < total:
        out.append((o, min(size, total - o)))
        o += size
    return out


def build_l1(D, cap):
    """Per-core expert FFN: H[d_out, n] = silu((1/WSCALE)*sum_k W[k,d_out]*XT[k,n] + be[d_out]).

    fp8e4 DoubleRow matmuls: XT is host-pretiled fp8 [C, P, K2, 2, 512]
    (k-tiles paired into DoubleRow groups) and W is fp8 [P, F, K2, 2, P]
    (partition-major so multi-f-tile slabs DMA as one transfer with long
    per-partition runs), prescaled by WSCALE on host.  W is fully
    SBUF-resident (4.2MB), XT streams through once.  H output is bf16,
    written in 4-f-tile batches [C, FQ, P, 4, 512] so each store DMA has
    4KB per-partition runs.

    DMA discipline: every dma_start costs ~0.6us of serial config time on
    the issuing engine's sequencer, so transfers are few and large.  All
    loads ride the SP (sync) HWDGE queue (no compute there); H stores ride
    the Activation queue, whose configs interleave naturally with the
    silus that produce them.
    """
    K2 = D // P // 2    # DoubleRow k-pairs
    F = D // P          # feat (d_out) tiles
    FQ = F // 4         # H store batches
    C = cap // 512      # 512-col chunks
    assert cap % 512 == 0
    nc = bacc.Bacc("TRN2", target_bir_lowering=False, debug=False)
    XT = nc.dram_tensor("XT", [C, P, K2, 2, 512], F8, kind="ExternalInput")
    W = nc.dram_tensor("W", [P, F, K2, 2, P], F8, kind="ExternalInput")
    BE = nc.dram_tensor("BE", [D], F32, kind="ExternalInput")
    H = nc.dram_tensor("H", [C, FQ, P, 4, 512], BF, kind="ExternalOutput")
    DR = mybir.MatmulPerfMode.DoubleRow

    with tile.TileContext(nc) as tc:
        with (
            tc.tile_pool(name="consts", bufs=1) as consts,
            tc.tile_pool(name="xt", bufs=3) as xtp,
            tc.tile_pool(name="hout", bufs=3) as hp,
            tc.tile_pool(name="ps", bufs=4, space="PSUM") as psp,
        ):
            # PE warm-up: ~48 tiny matmuls during the DMA lead-in keep the
            # HAM activity window busy so real matmuls start at 2.4GHz.
            warm = consts.tile([P, 64], BF)
            nc.vector.memset(warm[:], 1.0)
            wps = psp.tile([P, 512], F32, tag="ps", name="ps")
            for _ in range(36):
                nc.tensor.matmul(wps[0:64, 0:64], lhsT=warm[:, 0:64],
                                 rhs=warm[:], start=True, stop=True)

            # Two ~165GB/s HWDGE queues, loads split by need-time:
            # sync carries the XT stream + two W slabs, scalar carries W's
            # lead slabs (its configs precede all silu work in program
            # order) and then the H stores.  Slab sizes are paced against
            # the 1.8us/f-tile burn rate.
            w_all = consts.tile([P, F, K2, 2, P], F8)
            xt_tiles = {}
            xt_tiles[0] = xtp.tile([P, K2, 2, 512], F8, tag="xt", name="xt")
            nc.scalar.dma_start(w_all[:, 0], W[:, 0])
            nc.sync.dma_start(xt_tiles[0][:, 0:2], XT[0, :, 0:2])
            nc.scalar.dma_start(w_all[:, 1:3], W[:, 1:3])
            nc.sync.dma_start(xt_tiles[0][:, 2:4], XT[0, :, 2:4])
            nc.scalar.dma_start(w_all[:, 5:9], W[:, 5:9])
            nc.sync.dma_start(xt_tiles[0][:, 4 : K2], XT[0, :, 4 : K2])
            nc.scalar.dma_start(w_all[:, 11:F], W[:, 11:F])
            be_sb = consts.tile([P, F], F32)
            nc.sync.dma_start(be_sb[:], BE[:].rearrange("(f p) -> p f", p=P))
            nc.sync.dma_start(w_all[:, 3:5], W[:, 3:5])
            nc.sync.dma_start(w_all[:, 9:11], W[:, 9:11])
            for ci in range(1, C):
                xt_c = xtp.tile([P, K2, 2, 512], F8, tag="xt", name="xt")
                nc.sync.dma_start(xt_c[:], XT[ci])
                xt_tiles[ci] = xt_c

            for ci in range(C):
                xt_c = xt_tiles[ci]
                h4 = None
                for f in range(F):
                    if f % 4 == 0:
                        h4 = hp.tile([P, 4, 512], BF, tag="h4", name="h4")
                    ps = psp.tile([P, 512], F32, tag="ps", name="ps")
                    for k in range(K2):
                        nc.tensor.matmul(
                            ps[:],
                            lhsT=w_all[:, f, k],
                            rhs=xt_c[:, k],
                            start=(k == 0),
                            stop=(k == K2 - 1),
                            perf_mode=DR,
                        )
                    nc.scalar.activation(
                        h4[:, f % 4],
                        ps[:],
                        mybir.ActivationFunctionType.Silu,
                        bias=be_sb[:, f : f + 1],
                        scale=1.0 / WSCALE,
                    )
                    last_batch = ci == C - 1 and f // 4 == FQ - 1
                    if last_batch and f % 4 == 1:
                        # stream the final batch per-silu so the drain after
                        # the last silu is one f-tile on the idle sync queue
                        nc.scalar.dma_start(H[ci, f // 4, :, 0:2], h4[:, 0:2])
                    elif last_batch and f % 4 == 2:
                        nc.scalar.dma_start(H[ci, f // 4, :, 2:3], h4[:, 2:3])
                    elif last_batch and f % 4 == 3:
                        nc.sync.dma_start(H[ci, f // 4, :, 3:4], h4[:, 3:4])
                    elif f % 4 == 3:
                        nc.scalar.dma_start(H[ci, f // 4], h4[:])
    nc.compile()
    return nc


def build_l2(D, TPC, unit_nw, eps=1e-6):
    """Per-core combine + output proj + residual + RMS norm.

    Y[t, j] = nw[j] * (XIN[t,j] + sum_k CT[k,t]*Wo[k,j]) / rms(t)
    CT = g1*A + g2*B (host-combined, bf16); XIN = x_shard + bo (fp32).
    Y output is bf16 (host upcasts).

    The last n-chunk runs m-outer so each m's epilogue (RMS + scale +
    store) chains behind its own k-loop and overlaps the next m's
    matmuls; only m=M-1's epilogue trails the final matmul.

    unit_nw=True specializes norm_w == 1 (scale-by-rstd runs as a scalar
    engine activation; multiplying by 1 is exact) so the vector engine
    stays under the per-m tensor budget during the epilogue phase.
    """
    K = D // P
    M = TPC // P
    NC4 = D // 512
    KB = K // 4          # k-tiles bundled per DMA
    KB8 = 1              # leading k-bundles in fp8 (4 k-tiles = 2 DR pairs)
    KBF = KB - KB8       # remaining bf16 k-bundles
    KF = (K - 4 * KB8)   # bf16 k-tiles
    nc = bacc.Bacc("TRN2", target_bir_lowering=False, debug=False)
    # CT/WO are host-pretiled so every DMA reads long contiguous
    # per-partition runs — strided reads from the natural [D, x] layout
    # only sustain ~40% of DMA bandwidth and gate the lead-in.
    # The first 4 k-tiles run as fp8e4 DoubleRow matmuls (CT8/WO8),
    # accumulating into the same PSUM group as the bf16 remainder; all Wo
    # slices are prescaled x64 on host (exact in bf16) and the 1/64 is
    # folded into the psum-eviction op.  This shaves ~12% off L2's PE time
    # for ~0.2e-2 of extra rel err.
    # DMA discipline: dma_start costs ~0.6us serial config time on the
    # issuing engine, so transfers are few and large.  CT + XIN ride the
    # SP (sync) queue, WO rides the Activation queue (its prelude configs
    # precede all Act compute in program order).
    CT8 = nc.dram_tensor("CT8", [P, 2, 2, TPC], F8, kind="ExternalInput")
    CT = nc.dram_tensor("CT", [KBF, P, 4, TPC], BF, kind="ExternalInput")
    XIN = nc.dram_tensor("XIN", [TPC, D], BF, kind="ExternalInput")
    WO8 = nc.dram_tensor("WO8", [NC4, P, 2, 2, 512], F8, kind="ExternalInput")
    WO = nc.dram_tensor("WO", [NC4, KBF, P, 4, 512], BF, kind="ExternalInput")
    NW = nc.dram_tensor("NW", [D], F32, kind="ExternalInput")
    Y = nc.dram_tensor("Y", [TPC, D], BF, kind="ExternalOutput")

    XINr = XIN[:, :].rearrange("(m p) d -> p m d", p=P)
    DR = mybir.MatmulPerfMode.DoubleRow
    EV = 1.0 / WSCALE    # psum carries x64 products

    with tile.TileContext(nc) as tc:
        with (
            tc.tile_pool(name="consts", bufs=1) as consts,
            tc.tile_pool(name="ct", bufs=1) as ctp,
            tc.tile_pool(name="wo", bufs=3) as wop,
            tc.tile_pool(name="yall", bufs=1) as yallp,
            tc.tile_pool(name="sq", bufs=3) as sqp,
            tc.tile_pool(name="xin", bufs=2) as xinp,
            tc.tile_pool(name="yn", bufs=2) as ynp,
            tc.tile_pool(name="ssm", bufs=1) as ssmp,
            tc.tile_pool(name="stat", bufs=4) as statp,
            tc.tile_pool(name="ps", bufs=1, space="PSUM") as psp,
        ):
            # CT as one SBUF-resident tile; first bundle k-split so the
            # n=0 k-loop starts after ~0.4MB.
            # fp8 head is small (0.5MB CT8 + 0.25MB/chunk WO8): the first
            # matmuls gate on it; bf16 CT bundles stream behind on sync,
            # CT's last bundle rides the scalar queue between wo-n0 and
            # wo-n1 so both queues run flat out through the n=0 k-loop.
            ct8_all = ctp.tile([P, 2, 2, TPC], F8, tag="ct8", name="ct8")
            ct_all = ctp.tile([P, KBF, 4, TPC], BF, tag="ctbf", name="ctbf")
            WOr = [WO[n].rearrange("b p j x -> p b j x") for n in range(NC4)]
            wo8_cur = wop.tile([P, 2, 2, 512], F8, tag="wo8", name="wo8")
            wo_cur = wop.tile([P, KBF, 4, 512], BF, tag="wo", name="wo")
            # sync: fp8 head k-pair 0, then the bf16 CT bundles in
            # consumption order; scalar: WO n0 + fp8 head k-pair 1 + CT's
            # last bundle + WO n1 -- both queues ~paced to the n=0 k-loop.
            nc.sync.dma_start(ct8_all[:, 0], CT8[:, 0])
            nc.scalar.dma_start(wo8_cur[:], WO8[0])
            nc.scalar.dma_start(ct8_all[:, 1], CT8[:, 1])
            nc.sync.dma_start(ct_all[:, 0, 0:2], CT[0, :, 0:2])
            nc.scalar.dma_start(wo_cur[:, 0:1], WOr[0][:, 0:1])
            nc.sync.dma_start(ct_all[:, 0, 2:4], CT[0, :, 2:4])
            nc.scalar.dma_start(wo_cur[:, 1:KBF], WOr[0][:, 1:KBF])
            nc.sync.dma_start(ct_all[:, 1], CT[1])
            nc.scalar.dma_start(ct_all[:, 2], CT[2])
            wo8_nxt = wop.tile([P, 2, 2, 512], F8, tag="wo8", name="wo8")
            wo_nxt = wop.tile([P, KBF, 4, 512], BF, tag="wo", name="wo")
            nc.scalar.dma_start(wo8_nxt[:], WO8[1])
            nc.scalar.dma_start(wo_nxt[:], WOr[1])
            # y_all accumulator (fp32); the residual XIN streams in as
            # per-(m,n) bf16 slices added at psum-eviction time, so its
            # bytes never compete with the lead-in wo/ct stream.
            y_all = yallp.tile([P, M, D], F32)
            nw_sb = None
            if not unit_nw:
                nw_sb = consts.tile([P, D], F32)
                nwap = NW[:]
                nw_bcast = bass.AP(
                    tensor=nwap.tensor, offset=nwap.offset, ap=[[0, P]] + list(nwap.ap)
                )
                nc.sync.dma_start(nw_sb[:], nw_bcast)
            eps_sb = consts.tile([P, 1], F32)
            nc.vector.memset(eps_sb[:], eps)

            ssm_t = ssmp.tile([P, M], F32)
            ss_m = [ssm_t[:, m : m + 1] for m in range(M)]

            def stats(m, n, y_slice, first):
                # incremental RMS stats: ss_m[m] += sum(y_slice^2)
                ncols = y_slice.shape[-1]
                sq = sqp.tile([P, 1024], F32, tag="sq", name="sq")
                ssp = statp.tile([P, 1], F32, tag="ssp", name="ssp")
                nc.scalar.activation(
                    sq[:, :ncols],
                    y_slice,
                    mybir.ActivationFunctionType.Square,
                    accum_out=ssp[:],
                )
                if first:
                    nc.vector.tensor_copy(ss_m[m], ssp[:])
                else:
                    nc.vector.tensor_add(ss_m[m], ss_m[m], ssp[:])

            def epilogue(m):
                y_m = y_all[:, m, :]
                rms = statp.tile([P, 1], F32, tag="rms", name="rms")
                nc.scalar.activation(
                    rms[:],
                    ss_m[m],
                    mybir.ActivationFunctionType.Sqrt,
                    bias=eps_sb[:],
                    scale=1.0 / D,
                )
                rstd = statp.tile([P, 1], F32, tag="rstd", name="rstd")
                nc.vector.reciprocal(rstd[:], rms[:])
                yn = ynp.tile([P, D], BF, tag="yn", name="yn")
                if unit_nw and m == M - 1:
                    # final m: its scale trails the last matmul; pipeline
                    # 512-col pieces across scalar+vector and both HWDGE
                    # queues so the last Y write starts ASAP
                    for q in range(4):
                        qs = slice(q * 512, (q + 1) * 512)
                        if q % 2 == 0:
                            nc.scalar.activation(
                                yn[:, qs],
                                y_all[:, m, qs],
                                mybir.ActivationFunctionType.Identity,
                                bias=0.0,
                                scale=rstd[:],
                            )
                        else:
                            nc.vector.tensor_scalar_mul(
                                yn[:, qs], y_all[:, m, qs], rstd[:]
                            )
                        eng = nc.sync if q % 2 == 0 else nc.scalar
                        eng.dma_start(Y[m * P : (m + 1) * P, qs], yn[:, qs])
                    return
                for h in range(1):
                    hs = slice(0, D)
                    if unit_nw:
                        # non-final scales run on vector only (vector/m:
                        # evict+recip+scale ~2.3us < 3.0us tensor pace),
                        # keeping scalar clear for the final m's chain
                        nc.vector.tensor_scalar_mul(
                            yn[:, hs], y_all[:, m, hs], rstd[:]
                        )
                    else:
                        nc.vector.scalar_tensor_tensor(
                            yn[:, hs],
                            y_all[:, m, hs],
                            rstd[:],
                            nw_sb[:, hs],
                            op0=mybir.AluOpType.mult,
                            op1=mybir.AluOpType.mult,
                        )
                    nc.sync.dma_start(Y[m * P : (m + 1) * P, hs], yn[:, hs])

            pss = [psp.tile([P, 512], F32, tag=f"ps{m}", name=f"ps{m}")
                   for m in range(M)]

            # PE warm-up during the DMA lead-in (see build_l1)
            warm = consts.tile([P, 64], BF)
            nc.vector.memset(warm[:], 1.0)
            for _ in range(36):
                nc.tensor.matmul(pss[0][0:64, 0:64], lhsT=warm[:, 0:64],
                                 rhs=warm[:], start=True, stop=True)

            xin_prev = None
            for n in range(NC4):
                n0 = n * 512
                # one batched XIN load per n-chunk
                xin_n = xinp.tile([P, M, 512], BF, tag="xin", name="xin")
                nc.sync.dma_start(xin_n[:], XINr[:, :, n0 : n0 + 512])
                if n + 2 < NC4:
                    wo8_n2 = wop.tile([P, 2, 2, 512], F8, tag="wo8", name="wo8")
                    wo_n2 = wop.tile([P, KBF, 4, 512], BF, tag="wo", name="wo")
                    nc.scalar.dma_start(wo8_n2[:], WO8[n + 2])
                    nc.scalar.dma_start(wo_n2[:], WOr[n + 2])
                def mm_f8(m, kp, start):
                    nc.tensor.matmul(
                        pss[m][:],
                        lhsT=ct8_all[:, kp, :, m * P : (m + 1) * P],
                        rhs=wo8_cur[:, kp],
                        start=start,
                        stop=False,
                        perf_mode=DR,
                    )

                def mm_bf(m, k, stop):
                    nc.tensor.matmul(
                        pss[m][:],
                        lhsT=ct_all[:, k // 4, k % 4, m * P : (m + 1) * P],
                        rhs=wo_cur[:, k // 4, k % 4],
                        start=False,
                        stop=stop,
                    )

                if n == 0:
                    # k-outer for the first chunk: consumes each (wo, ct)
                    # bundle over 8 matmuls, pacing the k-loop to the DMA
                    # stream instead of stalling m=0 on the full 6MB.
                    for kp in range(2):
                        for m in range(M):
                            mm_f8(m, kp, kp == 0)
                    for k in range(KF):
                        for m in range(M):
                            mm_bf(m, k, k == KF - 1)
                    for m in range(M):
                        # defer the n=0 residual add: evict psum (x64) with
                        # a scaled copy so nothing here waits on XIN
                        nc.vector.tensor_scalar_mul(y_all[:, m, 0:512],
                                                    pss[m][:], EV)
                else:
                    # m-outer, k-inner: 16 consecutive matmuls accumulate
                    # into one PSUM bank before it's read (avoids psum-queue
                    # depth-cycling micro-idles).
                    for m in range(M):
                        for kp in range(2):
                            mm_f8(m, kp, kp == 0)
                        for k in range(KF):
                            mm_bf(m, k, k == KF - 1)
                        y_slice = y_all[:, m, n0 : n0 + 512]
                        # y = psum/64 + xin
                        nc.vector.scalar_tensor_tensor(
                            y_slice, pss[m][:], EV, xin_n[:, m],
                            op0=mybir.AluOpType.mult,
                            op1=mybir.AluOpType.add,
                        )
                        if n == 1:
                            # catch up n=0's deferred residual add and run
                            # stats over both chunks at once
                            nc.vector.tensor_add(
                                y_all[:, m, 0:512], y_all[:, m, 0:512],
                                xin_prev[:, m],
                            )
                            stats(m, n, y_all[:, m, 0:1024], first=True)
                        else:
                            stats(m, n, y_slice, first=False)
                        if n + 1 == NC4:
                            # chain each m's epilogue behind its own k-loop
                            # so only m=M-1's trails the final matmul
                            epilogue(m)
                xin_prev = xin_n
                if n + 1 < NC4:
                    wo_cur = wo_nxt
                    wo8_cur = wo8_nxt
                    if n + 2 < NC4:
                        wo_nxt = wo_n2
                        wo8_nxt = wo8_n2
    nc.compile()
    return nc


def host_dispatch(xf, Wr, br):
    """Router + top-2 + softmax gates + expert grouping. Returns dispatch info."""
    T, D = xf.shape
    E = Wr.shape[1]
    logits = xf @ Wr + br
    i1 = np.argmax(logits, axis=1)
    l2 = logits.copy()
    l2[np.arange(T), i1] = -np.inf
    i2 = np.argmax(l2, axis=1)
    v1 = logits[np.arange(T), i1]
    v2 = logits[np.arange(T), i2]
    e2 = np.exp(v2 - v1)
    g1 = (1.0 / (1.0 + e2)).astype(np.float32)
    g2 = (e2 / (1.0 + e2)).astype(np.float32)

    # flat slots (t,s) grouped by expert, stable by (token, slot)
    ee = np.stack([i1, i2], 1).ravel()          # [2T]
    gg = np.stack([g1, g2], 1).ravel()
    tt = np.repeat(np.arange(T), 2)
    order = np.argsort(ee, kind="stable")
    counts = np.bincount(ee, minlength=E)
    starts = np.concatenate([[0], np.cumsum(counts)[:-1]])
    rank = np.empty(2 * T, np.int64)
    rank[order] = np.arange(2 * T)
    pos = rank - starts[ee]                      # position within expert's list
    return dict(
        e1=i1, e2=i2, counts=counts, order=order, starts=starts,
        pos=pos.reshape(T, 2), tok=tt, gate=gg, g1=g1, g2=g2,
    )


def prep_l1_inputs(xf, d, We, be):
    """Per-expert L1 inputs: gathered+pretiled fp8 XT, fp8 W (x WSCALE),
    fp32 be.  Tokens past CAP (expert overflow) are evaluated here on host
    in fp32 and stashed in d["Hov"] for the combine.
    """
    T, D = xf.shape
    E = We.shape[0]
    K2 = D // P // 2
    F = D // P
    C = CAP // 512
    counts = d["counts"]
    We_f = np.asarray(We, np.float32)
    be_f = np.asarray(be, np.float32)
    in1 = []
    Hov = []
    for e in range(E):
        n_e = int(counts[e])
        sel = d["order"][d["starts"][e] : d["starts"][e] + n_e]
        toks = d["tok"][sel]
        n_dev = min(n_e, CAP)
        Xg = np.zeros((CAP, D), np.float32)
        Xg[:n_dev] = xf[toks[:n_dev]]
        # [C, P, K2, 2, n]: DoubleRow k-pair groups, contiguous 8KB
        # per-partition DMA runs
        XT_T = np.ascontiguousarray(
            Xg.astype(F8NP).reshape(C, 512, K2, 2, P).transpose(0, 4, 2, 3, 1)
        )
        # [P, F, K2, 2, P]: partition-major so f-tile slabs DMA as single
        # transfers with (4*f_span)KB per-partition runs
        W_T = np.ascontiguousarray(
            (We_f[e] * WSCALE).astype(F8NP)
            .reshape(K2, 2, P, F, P).transpose(2, 3, 0, 1, 4)
        )
        in1.append({"XT": XT_T, "W": W_T, "BE": be_f[e]})
        if n_e > CAP:
            Xov = xf[toks[CAP:]]                        # [m, D] fp32
            pre = Xov @ We_f[e] + be_f[e]
            ho = (pre / (1.0 + np.exp(-pre))).astype(np.float32)
            Hov.append(ho.T)                            # [D, m]
        else:
            Hov.append(np.zeros((D, 0), np.float32))
    d["Hov"] = Hov
    return in1, CAP


def prep_l2_inputs(xf, d, H, Wo, bo, norm_w):
    """Per-core L2 inputs. CT = g1*A + g2*B combined on host (fp32 math);
    the first 512 k-rows go out as fp8 (DoubleRow pairs), the rest bf16.
    All Wo slices are prescaled x64 (exact in bf16; lifts the fp8 rows out
    of e4m3's subnormal range).  XIN = x + bo in fp32."""
    T, D = xf.shape
    TPC = T // NCORE
    KB = D // P // 4
    KBF = KB - 1
    NC4 = D // 512
    Wo_f = np.asarray(Wo, np.float32) * WSCALE
    # fp8 head: [NC4, P, 2, 2, 512] DoubleRow k-pair groups
    Wo8_b = np.ascontiguousarray(
        Wo_f[:512].astype(F8NP)
        .reshape(2, 2, P, NC4, 512).transpose(3, 2, 0, 1, 4)
    )
    # bf16 tail: [NC4, KBF, P, 4, 512]
    Wo_b = np.ascontiguousarray(
        Wo_f[512:].astype(BF16)
        .reshape(KBF, 4, P, NC4, 512).transpose(3, 0, 2, 1, 4)
    )
    bo_f = np.asarray(bo, np.float32)
    nw_f = np.asarray(norm_w, np.float32)
    e1, e2, pos = d["e1"], d["e2"], d["pos"]
    g1, g2 = d["g1"], d["g2"]
    # device H (raw [C, FQ, P, 4, 512] batches -> [D, CAP], first CAP
    # slots) + host-computed overflow columns
    def unpack_h(h_raw):
        return np.ascontiguousarray(
            np.asarray(h_raw, np.float32).transpose(1, 3, 2, 0, 4).reshape(D, CAP)
        )
    Hfull = [np.concatenate([unpack_h(H[e]), d["Hov"][e]], axis=1)
             for e in range(H.shape[0])]
    in2 = []
    for c in range(NCORE):
        tl = np.arange(c * TPC, (c + 1) * TPC)
        CTf = np.empty((D, TPC), np.float32)
        BTf = np.empty((D, TPC), np.float32)
        for e in range(H.shape[0]):
            s1 = e1[tl] == e
            if s1.any():
                CTf[:, s1] = Hfull[e][:, pos[tl[s1], 0]]
            s2 = e2[tl] == e
            if s2.any():
                BTf[:, s2] = Hfull[e][:, pos[tl[s2], 1]]
        CTf = CTf * g1[tl][None, :] + BTf * g2[tl][None, :]
        CT8t = np.ascontiguousarray(
            CTf[:512].astype(F8NP).reshape(2, 2, P, TPC).transpose(2, 0, 1, 3)
        )
        CTt = np.ascontiguousarray(
            CTf[512:].reshape(KBF, 4, P, TPC).transpose(0, 2, 1, 3)
        ).astype(BF16)
        XIN = (xf[tl] + bo_f[None, :]).astype(BF16)
        in2.append({"CT8": CT8t, "CT": CTt, "XIN": XIN,
                    "WO8": Wo8_b, "WO": Wo_b, "NW": nw_f})
    return in2


# ----------------------------------------------------------------------------
# Harness entry point: full (unsharded) inputs -> full output.
# ----------------------------------------------------------------------------
_L1_CACHE = {}
_L2_CACHE = {}


def kernel(x, Wr, br, We, be, Wo, bo, norm_w):
    B, S, D = x.shape
    E = We.shape[0]
    T = B * S
    TPC = T // NCORE
    xf = np.ascontiguousarray(np.asarray(x, np.float32).reshape(T, D))
    d = host_dispatch(xf, np.asarray(Wr, np.float32), np.asarray(br, np.float32))

    in1, Bcap = prep_l1_inputs(xf, d, We, be)
    if (D, Bcap) not in _L1_CACHE:
        _L1_CACHE[(D, Bcap)] = build_l1(D, Bcap)
    r1 = run_bass_kernel_spmd(_L1_CACHE[(D, Bcap)], in1, list(range(NCORE)))
    H = np.stack([r1.results[e]["H"] for e in range(E)])

    in2 = prep_l2_inputs(xf, d, H, Wo, bo, norm_w)
    unit_nw = bool(np.all(np.asarray(norm_w, np.float32) == 1.0))
    if (D, TPC, unit_nw) not in _L2_CACHE:
        _L2_CACHE[(D, TPC, unit_nw)] = build_l2(D, TPC, unit_nw)
    r2 = run_bass_kernel_spmd(_L2_CACHE[(D, TPC, unit_nw)], in2, list(range(NCORE)))
    Y = np.concatenate([r2.results[c]["Y"] for c in range(NCORE)], axis=0)
    return Y.reshape(B, S, D).astype(np.asarray(x).dtype)



# revision 41
# speedup vs baseline: 1.7956x; 1.0145x over previous
# MoE EnhancedGatedFusion kernel for 8x TRN2 NeuronCores (expert-parallel).
#
# Decomposition:
#   host : router logits -> top2 -> softmax gates -> dispatch by expert
#   L1   : per-core (expert e): H_T[d_out, n] = silu(We[e].T-contract @ XT + be[e])
#          (ungated; fp8e4 DoubleRow matmuls, fp32 PSUM, bf16 H output)
#   host : combine CT = g1*A + g2*B during the token un-shuffle (the
#          "all-to-all return" glue), downcast once to bf16; tokens past the
#          per-expert capacity (CAP) are the standard MoE overflow case and
#          are evaluated on host in fp32 during the combine.
#   L2   : per-core (1024 tokens): out = CT.T @ Wo; y = XIN + out (XIN =
#          x_shard + bo folded on host, fp32); RMS-norm * norm_w.
#
# L1 matmuls run fp8e4 (e4m3) in DoubleRow perf mode: each instruction
# contracts 256 rows (two 128-deep k-groups) at 1 output column/cycle --
# measured 1.96x the bf16 MAC rate on hardware.  Weights are prescaled by
# 64 on host to clear the e4m3 subnormal range (std 0.022 -> 1.4) and the
# 1/64 is folded into the silu activation's input scale.  End-to-end rel
# err of the fp8 L1 + bf16 L2 pipeline is ~1.6e-2 (gate: 2e-2).
# L2 stays bf16: fp8 there would push total error past the gate.
import sys
import types

sys.path.insert(0, "/opt/trn_rl_repo")

import numpy as np
import ml_dtypes

BF16 = np.dtype(ml_dtypes.bfloat16)


def _install_ntff_hook():
    # antenv.axon_hooks is missing in this image; shim it so
    # run_bass_kernel_spmd(trace=True) can drive NTFF profiling.
    if "antenv.axon_hooks" in sys.modules:
        return
    try:
        from trn_agent_boot.trn_boot import _ntff_profile_via_ctypes

        hook = _ntff_profile_via_ctypes("/opt/axon/libaxon_pjrt.so")
    except Exception:
        hook = None
    mod = types.ModuleType("antenv.axon_hooks")
    mod.get_axon_ntff_profile_hook = lambda: hook
    mod.set_axon_ntff_profile_hook = lambda h: None
    sys.modules["antenv.axon_hooks"] = mod


_install_ntff_hook()

import concourse.bacc as bacc
import concourse.bass as bass
import concourse.tile as tile
from concourse import mybir
from concourse.bass_utils import run_bass_kernel_spmd

F32 = mybir.dt.float32
BF = mybir.dt.bfloat16
F8 = mybir.dt.float8e4
F8NP = np.dtype(ml_dtypes.float8_e4m3)  # TRN fp8e4: max normal 240
P = 128
NCORE = 8
CAP = 2048      # per-expert token capacity on device; overflow runs on host
WSCALE = 64.0   # We prescale before fp8 cast (undone in the silu's scale)


def _chunks(total, size):
    out = []
    o = 0
    while o < total:
        out.append((o, min(size, total - o)))
        o += size
    return out


def build_l1(D, cap):
    """Per-core expert FFN: H[d_out, n] = silu((1/WSCALE)*sum_k W[k,d_out]*XT[k,n] + be[d_out]).

    fp8e4 DoubleRow matmuls: XT is host-pretiled fp8 [C, P, K2, 2, 512]
    (k-tiles paired into DoubleRow groups) and W is fp8 [P, F, K2, 2, P]
    (partition-major so multi-f-tile slabs DMA as one transfer with long
    per-partition runs), prescaled by WSCALE on host.  W is fully
    SBUF-resident (4.2MB), XT streams through once.  H output is bf16,
    written in 4-f-tile batches [C, FQ, P, 4, 512] so each store DMA has
    4KB per-partition runs.

    DMA discipline: every dma_start costs ~0.6us of serial config time on
    the issuing engine's sequencer, so transfers are few and large.  All
    loads ride the SP (sync) HWDGE queue (no compute there); H stores ride
    the Activation queue, whose configs interleave naturally with the
    silus that produce them.
    """
    K2 = D // P // 2    # DoubleRow k-pairs
    F = D // P          # feat (d_out) tiles
    FQ = F // 4         # H store batches
    C = cap // 512      # 512-col chunks
    assert cap % 512 == 0
    nc = bacc.Bacc("TRN2", target_bir_lowering=False, debug=False)
    XT = nc.dram_tensor("XT", [C, P, K2, 2, 512], F8, kind="ExternalInput")
    W = nc.dram_tensor("W", [P, F, K2, 2, P], F8, kind="ExternalInput")
    # BE host-pretiled to [P, F]: the natural "(f p) -> p f" rearrange DMA
    # generates 2048 4-byte descriptors whose config alone takes 10us and
    # blocks the queue mid-lead-in
    BE = nc.dram_tensor("BE", [P, F], F32, kind="ExternalInput")
    H = nc.dram_tensor("H", [C, FQ, P, 4, 512], BF, kind="ExternalOutput")
    DR = mybir.MatmulPerfMode.DoubleRow

    with tile.TileContext(nc) as tc:
        with (
            tc.tile_pool(name="consts", bufs=1) as consts,
            tc.tile_pool(name="xt", bufs=3) as xtp,
            tc.tile_pool(name="hout", bufs=3) as hp,
            tc.tile_pool(name="ps", bufs=4, space="PSUM") as psp,
        ):
            # PE warm-up: ~48 tiny matmuls during the DMA lead-in keep the
            # HAM activity window busy so real matmuls start at 2.4GHz.
            warm = consts.tile([P, 64], BF)
            nc.vector.memset(warm[:], 1.0)
            wps = psp.tile([P, 512], F32, tag="ps", name="ps")
            for _ in range(36):
                nc.tensor.matmul(wps[0:64, 0:64], lhsT=warm[:, 0:64],
                                 rhs=warm[:], start=True, stop=True)

            # Two ~165GB/s HWDGE queues, loads split by need-time:
            # sync carries the XT stream + two W slabs, scalar carries W's
            # lead slabs (its configs precede all silu work in program
            # order) and then the H stores.  Slab sizes are paced against
            # the 1.8us/f-tile burn rate.
            w_all = consts.tile([P, F, K2, 2, P], F8)
            xt_tiles = {}
            xt_tiles[0] = xtp.tile([P, K2, 2, 512], F8, tag="xt", name="xt")
            nc.scalar.dma_start(w_all[:, 0], W[:, 0])
            nc.sync.dma_start(xt_tiles[0][:, 0:2], XT[0, :, 0:2])
            nc.scalar.dma_start(w_all[:, 1:3], W[:, 1:3])
            nc.sync.dma_start(xt_tiles[0][:, 2:4], XT[0, :, 2:4])
            nc.scalar.dma_start(w_all[:, 5:9], W[:, 5:9])
            nc.sync.dma_start(xt_tiles[0][:, 4 : K2], XT[0, :, 4 : K2])
            nc.scalar.dma_start(w_all[:, 11:F], W[:, 11:F])
            be_sb = consts.tile([P, F], F32)
            nc.sync.dma_start(be_sb[:], BE[:, :])
            nc.sync.dma_start(w_all[:, 3:5], W[:, 3:5])
            nc.sync.dma_start(w_all[:, 9:11], W[:, 9:11])
            for ci in range(1, C):
                xt_c = xtp.tile([P, K2, 2, 512], F8, tag="xt", name="xt")
                nc.sync.dma_start(xt_c[:], XT[ci])
                xt_tiles[ci] = xt_c

            for ci in range(C):
                xt_c = xt_tiles[ci]
                h4 = None
                for f in range(F):
                    if f % 4 == 0:
                        h4 = hp.tile([P, 4, 512], BF, tag="h4", name="h4")
                    ps = psp.tile([P, 512], F32, tag="ps", name="ps")
                    for k in range(K2):
                        nc.tensor.matmul(
                            ps[:],
                            lhsT=w_all[:, f, k],
                            rhs=xt_c[:, k],
                            start=(k == 0),
                            stop=(k == K2 - 1),
                            perf_mode=DR,
                        )
                    nc.scalar.activation(
                        h4[:, f % 4],
                        ps[:],
                        mybir.ActivationFunctionType.Silu,
                        bias=be_sb[:, f : f + 1],
                        scale=1.0 / WSCALE,
                    )
                    last_batch = ci == C - 1 and f // 4 == FQ - 1
                    if last_batch and f % 4 == 1:
                        # stream the final batch per-silu so the drain after
                        # the last silu is one f-tile on the idle sync queue
                        nc.scalar.dma_start(H[ci, f // 4, :, 0:2], h4[:, 0:2])
                    elif last_batch and f % 4 == 2:
                        nc.scalar.dma_start(H[ci, f // 4, :, 2:3], h4[:, 2:3])
                    elif last_batch and f % 4 == 3:
                        nc.sync.dma_start(H[ci, f // 4, :, 3:4], h4[:, 3:4])
                    elif f % 4 == 3:
                        nc.scalar.dma_start(H[ci, f // 4], h4[:])
    nc.compile()
    return nc


def build_l2(D, TPC, unit_nw, eps=1e-6):
    """Per-core combine + output proj + residual + RMS norm.

    Y[t, j] = nw[j] * (XIN[t,j] + sum_k CT[k,t]*Wo[k,j]) / rms(t)
    CT = g1*A + g2*B (host-combined, bf16); XIN = x_shard + bo (fp32).
    Y output is bf16 (host upcasts).

    The last n-chunk runs m-outer so each m's epilogue (RMS + scale +
    store) chains behind its own k-loop and overlaps the next m's
    matmuls; only m=M-1's epilogue trails the final matmul.

    unit_nw=True specializes norm_w == 1 (scale-by-rstd runs as a scalar
    engine activation; multiplying by 1 is exact) so the vector engine
    stays under the per-m tensor budget during the epilogue phase.
    """
    K = D // P
    M = TPC // P
    NC4 = D // 512
    KB = K // 4          # k-tiles bundled per DMA
    KB8 = 1              # leading k-bundles in fp8 (4 k-tiles = 2 DR pairs)
    KBF = KB - KB8       # remaining bf16 k-bundles
    KF = (K - 4 * KB8)   # bf16 k-tiles
    nc = bacc.Bacc("TRN2", target_bir_lowering=False, debug=False)
    # CT/WO are host-pretiled so every DMA reads long contiguous
    # per-partition runs — strided reads from the natural [D, x] layout
    # only sustain ~40% of DMA bandwidth and gate the lead-in.
    # The first 4 k-tiles run as fp8e4 DoubleRow matmuls (CT8/WO8),
    # accumulating into the same PSUM group as the bf16 remainder; all Wo
    # slices are prescaled x64 on host (exact in bf16) and the 1/64 is
    # folded into the psum-eviction op.  This shaves ~12% off L2's PE time
    # for ~0.2e-2 of extra rel err.
    # DMA discipline: dma_start costs ~0.6us serial config time on the
    # issuing engine, so transfers are few and large.  CT + XIN ride the
    # SP (sync) queue, WO rides the Activation queue (its prelude configs
    # precede all Act compute in program order).
    CT8 = nc.dram_tensor("CT8", [P, 2, 2, TPC], F8, kind="ExternalInput")
    CT = nc.dram_tensor("CT", [KBF, P, 4, TPC], BF, kind="ExternalInput")
    XIN = nc.dram_tensor("XIN", [TPC, D], BF, kind="ExternalInput")
    WO8 = nc.dram_tensor("WO8", [NC4, P, 2, 2, 512], F8, kind="ExternalInput")
    WO = nc.dram_tensor("WO", [NC4, KBF, P, 4, 512], BF, kind="ExternalInput")
    NW = nc.dram_tensor("NW", [D], F32, kind="ExternalInput")
    Y = nc.dram_tensor("Y", [TPC, D], BF, kind="ExternalOutput")

    XINr = XIN[:, :].rearrange("(m p) d -> p m d", p=P)
    DR = mybir.MatmulPerfMode.DoubleRow
    EV = 1.0 / WSCALE    # psum carries x64 products

    with tile.TileContext(nc) as tc:
        with (
            tc.tile_pool(name="consts", bufs=1) as consts,
            tc.tile_pool(name="ct", bufs=1) as ctp,
            tc.tile_pool(name="wo", bufs=3) as wop,
            tc.tile_pool(name="yall", bufs=1) as yallp,
            tc.tile_pool(name="sq", bufs=3) as sqp,
            tc.tile_pool(name="xin", bufs=2) as xinp,
            tc.tile_pool(name="yn", bufs=2) as ynp,
            tc.tile_pool(name="ssm", bufs=1) as ssmp,
            tc.tile_pool(name="stat", bufs=4) as statp,
            tc.tile_pool(name="ps", bufs=1, space="PSUM") as psp,
        ):
            # CT as one SBUF-resident tile; first bundle k-split so the
            # n=0 k-loop starts after ~0.4MB.
            # fp8 head is small (0.5MB CT8 + 0.25MB/chunk WO8): the first
            # matmuls gate on it; bf16 CT bundles stream behind on sync,
            # CT's last bundle rides the scalar queue between wo-n0 and
            # wo-n1 so both queues run flat out through the n=0 k-loop.
            ct8_all = ctp.tile([P, 2, 2, TPC], F8, tag="ct8", name="ct8")
            ct_all = ctp.tile([P, KBF, 4, TPC], BF, tag="ctbf", name="ctbf")
            WOr = [WO[n].rearrange("b p j x -> p b j x") for n in range(NC4)]
            wo8_cur = wop.tile([P, 2, 2, 512], F8, tag="wo8", name="wo8")
            wo_cur = wop.tile([P, KBF, 4, 512], BF, tag="wo", name="wo")
            # sync: fp8 head k-pair 0, then the bf16 CT bundles in
            # consumption order; scalar: WO n0 + fp8 head k-pair 1 + CT's
            # last bundle + WO n1 -- both queues ~paced to the n=0 k-loop.
            nc.sync.dma_start(ct8_all[:, 0], CT8[:, 0])
            nc.scalar.dma_start(wo8_cur[:], WO8[0])
            nc.scalar.dma_start(ct8_all[:, 1], CT8[:, 1])
            nc.sync.dma_start(ct_all[:, 0, 0:2], CT[0, :, 0:2])
            nc.scalar.dma_start(wo_cur[:, 0:1], WOr[0][:, 0:1])
            nc.sync.dma_start(ct_all[:, 0, 2:4], CT[0, :, 2:4])
            nc.scalar.dma_start(wo_cur[:, 1:KBF], WOr[0][:, 1:KBF])
            nc.sync.dma_start(ct_all[:, 1], CT[1])
            nc.scalar.dma_start(ct_all[:, 2], CT[2])
            wo8_nxt = wop.tile([P, 2, 2, 512], F8, tag="wo8", name="wo8")
            wo_nxt = wop.tile([P, KBF, 4, 512], BF, tag="wo", name="wo")
            nc.scalar.dma_start(wo8_nxt[:], WO8[1])
            nc.scalar.dma_start(wo_nxt[:], WOr[1])
            # y_all accumulator (fp32); the residual XIN streams in as
            # per-(m,n) bf16 slices added at psum-eviction time, so its
            # bytes never compete with the lead-in wo/ct stream.
            y_all = yallp.tile([P, M, D], F32)
            nw_sb = None
            if not unit_nw:
                nw_sb = consts.tile([P, D], F32)
                nwap = NW[:]
                nw_bcast = bass.AP(
                    tensor=nwap.tensor, offset=nwap.offset, ap=[[0, P]] + list(nwap.ap)
                )
                nc.sync.dma_start(nw_sb[:], nw_bcast)
            eps_sb = consts.tile([P, 1], F32)
            nc.vector.memset(eps_sb[:], eps)

            ssm_t = ssmp.tile([P, M], F32)
            ss_m = [ssm_t[:, m : m + 1] for m in range(M)]

            def stats(m, n, y_slice, first):
                # incremental RMS stats: ss_m[m] += sum(y_slice^2)
                ncols = y_slice.shape[-1]
                sq = sqp.tile([P, 1024], F32, tag="sq", name="sq")
                ssp = statp.tile([P, 1], F32, tag="ssp", name="ssp")
                nc.scalar.activation(
                    sq[:, :ncols],
                    y_slice,
                    mybir.ActivationFunctionType.Square,
                    accum_out=ssp[:],
                )
                if first:
                    nc.vector.tensor_copy(ss_m[m], ssp[:])
                else:
                    nc.vector.tensor_add(ss_m[m], ss_m[m], ssp[:])

            def epilogue(m):
                y_m = y_all[:, m, :]
                rms = statp.tile([P, 1], F32, tag="rms", name="rms")
                nc.scalar.activation(
                    rms[:],
                    ss_m[m],
                    mybir.ActivationFunctionType.Sqrt,
                    bias=eps_sb[:],
                    scale=1.0 / D,
                )
                rstd = statp.tile([P, 1], F32, tag="rstd", name="rstd")
                nc.vector.reciprocal(rstd[:], rms[:])
                yn = ynp.tile([P, D], BF, tag="yn", name="yn")
                if unit_nw and m == M - 1:
                    # final m: its scale trails the last matmul; pipeline
                    # 512-col pieces across scalar+vector and both HWDGE
                    # queues so the last Y write starts ASAP
                    for q in range(4):
                        qs = slice(q * 512, (q + 1) * 512)
                        if q % 2 == 0:
                            nc.scalar.activation(
                                yn[:, qs],
                                y_all[:, m, qs],
                                mybir.ActivationFunctionType.Identity,
                                bias=0.0,
                                scale=rstd[:],
                            )
                        else:
                            nc.vector.tensor_scalar_mul(
                                yn[:, qs], y_all[:, m, qs], rstd[:]
                            )
                        eng = nc.sync if q % 2 == 0 else nc.scalar
                        eng.dma_start(Y[m * P : (m + 1) * P, qs], yn[:, qs])
                    return
                for h in range(1):
                    hs = slice(0, D)
                    if unit_nw:
                        # alternate scale engines by m: vector also carries
                        # the evicts/recips, scalar the squares/sqrts, so
                        # neither backlogs behind the 3.0us/m tensor pace
                        if m % 2 == 1:
                            nc.scalar.activation(
                                yn[:, hs],
                                y_all[:, m, hs],
                                mybir.ActivationFunctionType.Identity,
                                bias=0.0,
                                scale=rstd[:],
                            )
                        else:
                            nc.vector.tensor_scalar_mul(
                                yn[:, hs], y_all[:, m, hs], rstd[:]
                            )
                    else:
                        nc.vector.scalar_tensor_tensor(
                            yn[:, hs],
                            y_all[:, m, hs],
                            rstd[:],
                            nw_sb[:, hs],
                            op0=mybir.AluOpType.mult,
                            op1=mybir.AluOpType.mult,
                        )
                    nc.sync.dma_start(Y[m * P : (m + 1) * P, hs], yn[:, hs])

            pss = [psp.tile([P, 512], F32, tag=f"ps{m}", name=f"ps{m}")
                   for m in range(M)]

            # PE warm-up during the DMA lead-in (see build_l1)
            warm = consts.tile([P, 64], BF)
            nc.vector.memset(warm[:], 1.0)
            for _ in range(36):
                nc.tensor.matmul(pss[0][0:64, 0:64], lhsT=warm[:, 0:64],
                                 rhs=warm[:], start=True, stop=True)

            xin_prev = None
            for n in range(NC4):
                n0 = n * 512
                # one batched XIN load per n-chunk
                xin_n = xinp.tile([P, M, 512], BF, tag="xin", name="xin")
                nc.sync.dma_start(xin_n[:], XINr[:, :, n0 : n0 + 512])
                if n + 2 < NC4:
                    wo8_n2 = wop.tile([P, 2, 2, 512], F8, tag="wo8", name="wo8")
                    wo_n2 = wop.tile([P, KBF, 4, 512], BF, tag="wo", name="wo")
                    nc.scalar.dma_start(wo8_n2[:], WO8[n + 2])
                    nc.scalar.dma_start(wo_n2[:], WOr[n + 2])
                def mm_f8(m, kp, start):
                    nc.tensor.matmul(
                        pss[m][:],
                        lhsT=ct8_all[:, kp, :, m * P : (m + 1) * P],
                        rhs=wo8_cur[:, kp],
                        start=start,
                        stop=False,
                        perf_mode=DR,
                    )

                def mm_bf(m, k, stop):
                    nc.tensor.matmul(
                        pss[m][:],
                        lhsT=ct_all[:, k // 4, k % 4, m * P : (m + 1) * P],
                        rhs=wo_cur[:, k // 4, k % 4],
                        start=False,
                        stop=stop,
                    )

                if n == 0:
                    # k-outer for the first chunk: consumes each (wo, ct)
                    # bundle over 8 matmuls, pacing the k-loop to the DMA
                    # stream instead of stalling m=0 on the full 6MB.
                    for kp in range(2):
                        for m in range(M):
                            mm_f8(m, kp, kp == 0)
                    for k in range(KF):
                        for m in range(M):
                            mm_bf(m, k, k == KF - 1)
                    for m in range(M):
                        # defer the n=0 residual add: evict psum (x64) with
                        # a scaled copy so nothing here waits on XIN
                        nc.vector.tensor_scalar_mul(y_all[:, m, 0:512],
                                                    pss[m][:], EV)
                else:
                    # m-outer, k-inner: 16 consecutive matmuls accumulate
                    # into one PSUM bank before it's read (avoids psum-queue
                    # depth-cycling micro-idles).
                    for m in range(M):
                        for kp in range(2):
                            mm_f8(m, kp, kp == 0)
                        for k in range(KF):
                            mm_bf(m, k, k == KF - 1)
                        y_slice = y_all[:, m, n0 : n0 + 512]
                        # y = psum/64 + xin
                        nc.vector.scalar_tensor_tensor(
                            y_slice, pss[m][:], EV, xin_n[:, m],
                            op0=mybir.AluOpType.mult,
                            op1=mybir.AluOpType.add,
                        )
                        if n == 1:
                            # catch up n=0's deferred residual add and run
                            # stats over both chunks at once
                            nc.vector.tensor_add(
                                y_all[:, m, 0:512], y_all[:, m, 0:512],
                                xin_prev[:, m],
                            )
                            stats(m, n, y_all[:, m, 0:1024], first=True)
                        else:
                            stats(m, n, y_slice, first=False)
                        if n + 1 == NC4:
                            # chain each m's epilogue behind its own k-loop
                            # so only m=M-1's trails the final matmul
                            epilogue(m)
                xin_prev = xin_n
                if n + 1 < NC4:
                    wo_cur = wo_nxt
                    wo8_cur = wo8_nxt
                    if n + 2 < NC4:
                        wo_nxt = wo_n2
                        wo8_nxt = wo8_n2
    nc.compile()
    return nc


def host_dispatch(xf, Wr, br):
    """Router + top-2 + softmax gates + expert grouping. Returns dispatch info."""
    T, D = xf.shape
    E = Wr.shape[1]
    logits = xf @ Wr + br
    i1 = np.argmax(logits, axis=1)
    l2 = logits.copy()
    l2[np.arange(T), i1] = -np.inf
    i2 = np.argmax(l2, axis=1)
    v1 = logits[np.arange(T), i1]
    v2 = logits[np.arange(T), i2]
    e2 = np.exp(v2 - v1)
    g1 = (1.0 / (1.0 + e2)).astype(np.float32)
    g2 = (e2 / (1.0 + e2)).astype(np.float32)

    # flat slots (t,s) grouped by expert, stable by (token, slot)
    ee = np.stack([i1, i2], 1).ravel()          # [2T]
    gg = np.stack([g1, g2], 1).ravel()
    tt = np.repeat(np.arange(T), 2)
    order = np.argsort(ee, kind="stable")
    counts = np.bincount(ee, minlength=E)
    starts = np.concatenate([[0], np.cumsum(counts)[:-1]])
    rank = np.empty(2 * T, np.int64)
    rank[order] = np.arange(2 * T)
    pos = rank - starts[ee]                      # position within expert's list
    return dict(
        e1=i1, e2=i2, counts=counts, order=order, starts=starts,
        pos=pos.reshape(T, 2), tok=tt, gate=gg, g1=g1, g2=g2,
    )


def prep_l1_inputs(xf, d, We, be):
    """Per-expert L1 inputs: gathered+pretiled fp8 XT, fp8 W (x WSCALE),
    fp32 be.  Tokens past CAP (expert overflow) are evaluated here on host
    in fp32 and stashed in d["Hov"] for the combine.
    """
    T, D = xf.shape
    E = We.shape[0]
    K2 = D // P // 2
    F = D // P
    C = CAP // 512
    counts = d["counts"]
    We_f = np.asarray(We, np.float32)
    be_f = np.asarray(be, np.float32)
    in1 = []
    Hov = []
    for e in range(E):
        n_e = int(counts[e])
        sel = d["order"][d["starts"][e] : d["starts"][e] + n_e]
        toks = d["tok"][sel]
        n_dev = min(n_e, CAP)
        Xg = np.zeros((CAP, D), np.float32)
        Xg[:n_dev] = xf[toks[:n_dev]]
        # [C, P, K2, 2, n]: DoubleRow k-pair groups, contiguous 8KB
        # per-partition DMA runs
        XT_T = np.ascontiguousarray(
            Xg.astype(F8NP).reshape(C, 512, K2, 2, P).transpose(0, 4, 2, 3, 1)
        )
        # [P, F, K2, 2, P]: partition-major so f-tile slabs DMA as single
        # transfers with (4*f_span)KB per-partition runs
        W_T = np.ascontiguousarray(
            (We_f[e] * WSCALE).astype(F8NP)
            .reshape(K2, 2, P, F, P).transpose(2, 3, 0, 1, 4)
        )
        in1.append({"XT": XT_T, "W": W_T,
                    "BE": np.ascontiguousarray(be_f[e].reshape(F, P).T)})
        if n_e > CAP:
            Xov = xf[toks[CAP:]]                        # [m, D] fp32
            pre = Xov @ We_f[e] + be_f[e]
            ho = (pre / (1.0 + np.exp(-pre))).astype(np.float32)
            Hov.append(ho.T)                            # [D, m]
        else:
            Hov.append(np.zeros((D, 0), np.float32))
    d["Hov"] = Hov
    return in1, CAP


def prep_l2_inputs(xf, d, H, Wo, bo, norm_w):
    """Per-core L2 inputs. CT = g1*A + g2*B combined on host (fp32 math);
    the first 512 k-rows go out as fp8 (DoubleRow pairs), the rest bf16.
    All Wo slices are prescaled x64 (exact in bf16; lifts the fp8 rows out
    of e4m3's subnormal range).  XIN = x + bo in fp32."""
    T, D = xf.shape
    TPC = T // NCORE
    KB = D // P // 4
    KBF = KB - 1
    NC4 = D // 512
    Wo_f = np.asarray(Wo, np.float32) * WSCALE
    # fp8 head: [NC4, P, 2, 2, 512] DoubleRow k-pair groups
    Wo8_b = np.ascontiguousarray(
        Wo_f[:512].astype(F8NP)
        .reshape(2, 2, P, NC4, 512).transpose(3, 2, 0, 1, 4)
    )
    # bf16 tail: [NC4, KBF, P, 4, 512]
    Wo_b = np.ascontiguousarray(
        Wo_f[512:].astype(BF16)
        .reshape(KBF, 4, P, NC4, 512).transpose(3, 0, 2, 1, 4)
    )
    bo_f = np.asarray(bo, np.float32)
    nw_f = np.asarray(norm_w, np.float32)
    e1, e2, pos = d["e1"], d["e2"], d["pos"]
    g1, g2 = d["g1"], d["g2"]
    # device H (raw [C, FQ, P, 4, 512] batches -> [D, CAP], first CAP
    # slots) + host-computed overflow columns
    def unpack_h(h_raw):
        return np.ascontiguousarray(
            np.asarray(h_raw, np.float32).transpose(1, 3, 2, 0, 4).reshape(D, CAP)
        )
    Hfull = [np.concatenate([unpack_h(H[e]), d["Hov"][e]], axis=1)
             for e in range(H.shape[0])]
    in2 = []
    for c in range(NCORE):
        tl = np.arange(c * TPC, (c + 1) * TPC)
        CTf = np.empty((D, TPC), np.float32)
        BTf = np.empty((D, TPC), np.float32)
        for e in range(H.shape[0]):
            s1 = e1[tl] == e
            if s1.any():
                CTf[:, s1] = Hfull[e][:, pos[tl[s1], 0]]
            s2 = e2[tl] == e
            if s2.any():
                BTf[:, s2] = Hfull[e][:, pos[tl[s2], 1]]
        CTf = CTf * g1[tl][None, :] + BTf * g2[tl][None, :]
        CT8t = np.ascontiguousarray(
            CTf[:512].astype(F8NP).reshape(2, 2, P, TPC).transpose(2, 0, 1, 3)
        )
        CTt = np.ascontiguousarray(
            CTf[512:].reshape(KBF, 4, P, TPC).transpose(0, 2, 1, 3)
        ).astype(BF16)
        XIN = (xf[tl] + bo_f[None, :]).astype(BF16)
        in2.append({"CT8": CT8t, "CT": CTt, "XIN": XIN,
                    "WO8": Wo8_b, "WO": Wo_b, "NW": nw_f})
    return in2


# ----------------------------------------------------------------------------
# Harness entry point: full (unsharded) inputs -> full output.
# ----------------------------------------------------------------------------
_L1_CACHE = {}
_L2_CACHE = {}


def kernel(x, Wr, br, We, be, Wo, bo, norm_w):
    B, S, D = x.shape
    E = We.shape[0]
    T = B * S
    TPC = T // NCORE
    xf = np.ascontiguousarray(np.asarray(x, np.float32).reshape(T, D))
    d = host_dispatch(xf, np.asarray(Wr, np.float32), np.asarray(br, np.float32))

    in1, Bcap = prep_l1_inputs(xf, d, We, be)
    if (D, Bcap) not in _L1_CACHE:
        _L1_CACHE[(D, Bcap)] = build_l1(D, Bcap)
    r1 = run_bass_kernel_spmd(_L1_CACHE[(D, Bcap)], in1, list(range(NCORE)))
    H = np.stack([r1.results[e]["H"] for e in range(E)])

    in2 = prep_l2_inputs(xf, d, H, Wo, bo, norm_w)
    unit_nw = bool(np.all(np.asarray(norm_w, np.float32) == 1.0))
    if (D, TPC, unit_nw) not in _L2_CACHE:
        _L2_CACHE[(D, TPC, unit_nw)] = build_l2(D, TPC, unit_nw)
    r2 = run_bass_kernel_spmd(_L2_CACHE[(D, TPC, unit_nw)], in2, list(range(NCORE)))
    Y = np.concatenate([r2.results[c]["Y"] for c in range(NCORE)], axis=0)
    return Y.reshape(B, S, D).astype(np.asarray(x).dtype)



# revision 43
# speedup vs baseline: 1.8048x; 1.0052x over previous
# MoE EnhancedGatedFusion kernel for 8x TRN2 NeuronCores (expert-parallel).
#
# Decomposition:
#   host : router logits -> top2 -> softmax gates -> dispatch by expert
#   L1   : per-core (expert e): H_T[d_out, n] = silu(We[e].T-contract @ XT + be[e])
#          (ungated; fp8e4 DoubleRow matmuls, fp32 PSUM, bf16 H output)
#   host : combine CT = g1*A + g2*B during the token un-shuffle (the
#          "all-to-all return" glue), downcast once to bf16; tokens past the
#          per-expert capacity (CAP) are the standard MoE overflow case and
#          are evaluated on host in fp32 during the combine.
#   L2   : per-core (1024 tokens): out = CT.T @ Wo; y = XIN + out (XIN =
#          x_shard + bo folded on host, fp32); RMS-norm * norm_w.
#
# L1 matmuls run fp8e4 (e4m3) in DoubleRow perf mode: each instruction
# contracts 256 rows (two 128-deep k-groups) at 1 output column/cycle --
# measured 1.96x the bf16 MAC rate on hardware.  Weights are prescaled by
# 64 on host to clear the e4m3 subnormal range (std 0.022 -> 1.4) and the
# 1/64 is folded into the silu activation's input scale.  End-to-end rel
# err of the fp8 L1 + bf16 L2 pipeline is ~1.6e-2 (gate: 2e-2).
# L2 stays bf16: fp8 there would push total error past the gate.
import sys
import types

sys.path.insert(0, "/opt/trn_rl_repo")

import numpy as np
import ml_dtypes

BF16 = np.dtype(ml_dtypes.bfloat16)


def _install_ntff_hook():
    # antenv.axon_hooks is missing in this image; shim it so
    # run_bass_kernel_spmd(trace=True) can drive NTFF profiling.
    if "antenv.axon_hooks" in sys.modules:
        return
    try:
        from trn_agent_boot.trn_boot import _ntff_profile_via_ctypes

        hook = _ntff_profile_via_ctypes("/opt/axon/libaxon_pjrt.so")
    except Exception:
        hook = None
    mod = types.ModuleType("antenv.axon_hooks")
    mod.get_axon_ntff_profile_hook = lambda: hook
    mod.set_axon_ntff_profile_hook = lambda h: None
    sys.modules["antenv.axon_hooks"] = mod


_install_ntff_hook()

import concourse.bacc as bacc
import concourse.bass as bass
import concourse.tile as tile
from concourse import mybir
from concourse.bass_utils import run_bass_kernel_spmd

F32 = mybir.dt.float32
BF = mybir.dt.bfloat16
F8 = mybir.dt.float8e4
F8NP = np.dtype(ml_dtypes.float8_e4m3)  # TRN fp8e4: max normal 240
P = 128
NCORE = 8
CAP = 2048      # per-expert token capacity on device; overflow runs on host
WSCALE = 64.0   # We prescale before fp8 cast (undone in the silu's scale)


def _chunks(total, size):
    out = []
    o = 0
    while o < total:
        out.append((o, min(size, total - o)))
        o += size
    return out


def build_l1(D, cap):
    """Per-core expert FFN: H[d_out, n] = silu((1/WSCALE)*sum_k W[k,d_out]*XT[k,n] + be[d_out]).

    fp8e4 DoubleRow matmuls: XT is host-pretiled fp8 [C, P, K2, 2, 512]
    (k-tiles paired into DoubleRow groups) and W is fp8 [P, F, K2, 2, P]
    (partition-major so multi-f-tile slabs DMA as one transfer with long
    per-partition runs), prescaled by WSCALE on host.  W is fully
    SBUF-resident (4.2MB), XT streams through once.  H output is bf16,
    written in 4-f-tile batches [C, FQ, P, 4, 512] so each store DMA has
    4KB per-partition runs.

    DMA discipline: every dma_start costs ~0.6us of serial config time on
    the issuing engine's sequencer, so transfers are few and large.  All
    loads ride the SP (sync) HWDGE queue (no compute there); H stores ride
    the Activation queue, whose configs interleave naturally with the
    silus that produce them.
    """
    K2 = D // P // 2    # DoubleRow k-pairs
    F = D // P          # feat (d_out) tiles
    FQ = F // 4         # H store batches
    C = cap // 512      # 512-col chunks
    assert cap % 512 == 0
    nc = bacc.Bacc("TRN2", target_bir_lowering=False, debug=False)
    XT = nc.dram_tensor("XT", [C, P, K2, 2, 512], F8, kind="ExternalInput")
    W = nc.dram_tensor("W", [P, F, K2, 2, P], F8, kind="ExternalInput")
    # BE host-pretiled to [P, F]: the natural "(f p) -> p f" rearrange DMA
    # generates 2048 4-byte descriptors whose config alone takes 10us and
    # blocks the queue mid-lead-in
    BE = nc.dram_tensor("BE", [P, F], F32, kind="ExternalInput")
    H = nc.dram_tensor("H", [C, FQ, P, 4, 512], BF, kind="ExternalOutput")
    DR = mybir.MatmulPerfMode.DoubleRow

    with tile.TileContext(nc) as tc:
        with (
            tc.tile_pool(name="consts", bufs=1) as consts,
            tc.tile_pool(name="xt", bufs=3) as xtp,
            tc.tile_pool(name="hout", bufs=3) as hp,
            tc.tile_pool(name="ps", bufs=4, space="PSUM") as psp,
        ):
            # PE warm-up: ~48 tiny matmuls during the DMA lead-in keep the
            # HAM activity window busy so real matmuls start at 2.4GHz.
            warm = consts.tile([P, 64], BF)
            nc.vector.memset(warm[:], 1.0)
            wps = psp.tile([P, 512], F32, tag="ps", name="ps")
            for _ in range(36):
                nc.tensor.matmul(wps[0:64, 0:64], lhsT=warm[:, 0:64],
                                 rhs=warm[:], start=True, stop=True)

            # Two ~165GB/s HWDGE queues, loads split by need-time:
            # sync carries the XT stream + two W slabs, scalar carries W's
            # lead slabs (its configs precede all silu work in program
            # order) and then the H stores.  Slab sizes are paced against
            # the 1.8us/f-tile burn rate.
            w_all = consts.tile([P, F, K2, 2, P], F8)
            xt_tiles = {}
            xt_tiles[0] = xtp.tile([P, K2, 2, 512], F8, tag="xt", name="xt")
            nc.scalar.dma_start(w_all[:, 0], W[:, 0])
            nc.sync.dma_start(xt_tiles[0][:, 0:2], XT[0, :, 0:2])
            nc.scalar.dma_start(w_all[:, 1:3], W[:, 1:3])
            nc.sync.dma_start(xt_tiles[0][:, 2:4], XT[0, :, 2:4])
            nc.scalar.dma_start(w_all[:, 3:5], W[:, 3:5])
            nc.sync.dma_start(xt_tiles[0][:, 4 : K2], XT[0, :, 4 : K2])
            nc.scalar.dma_start(w_all[:, 5:7], W[:, 5:7])
            be_sb = consts.tile([P, F], F32)
            nc.sync.dma_start(be_sb[:], BE[:, :])
            nc.scalar.dma_start(w_all[:, 7:9], W[:, 7:9])
            nc.sync.dma_start(w_all[:, 9:11], W[:, 9:11])
            nc.sync.dma_start(w_all[:, 11:13], W[:, 11:13])
            xt_tiles[1] = xtp.tile([P, K2, 2, 512], F8, tag="xt", name="xt")
            nc.sync.dma_start(xt_tiles[1][:], XT[1])
            nc.sync.dma_start(w_all[:, 13:F], W[:, 13:F])
            for ci in range(2, C):
                xt_c = xtp.tile([P, K2, 2, 512], F8, tag="xt", name="xt")
                nc.sync.dma_start(xt_c[:], XT[ci])
                xt_tiles[ci] = xt_c

            for ci in range(C):
                xt_c = xt_tiles[ci]
                h4 = None
                for f in range(F):
                    if f % 4 == 0:
                        h4 = hp.tile([P, 4, 512], BF, tag="h4", name="h4")
                    ps = psp.tile([P, 512], F32, tag="ps", name="ps")
                    for k in range(K2):
                        nc.tensor.matmul(
                            ps[:],
                            lhsT=w_all[:, f, k],
                            rhs=xt_c[:, k],
                            start=(k == 0),
                            stop=(k == K2 - 1),
                            perf_mode=DR,
                        )
                    nc.scalar.activation(
                        h4[:, f % 4],
                        ps[:],
                        mybir.ActivationFunctionType.Silu,
                        bias=be_sb[:, f : f + 1],
                        scale=1.0 / WSCALE,
                    )
                    last_batch = ci == C - 1 and f // 4 == FQ - 1
                    if last_batch and f % 4 == 1:
                        # stream the final batch per-silu so the drain after
                        # the last silu is one f-tile on the idle sync queue
                        nc.scalar.dma_start(H[ci, f // 4, :, 0:2], h4[:, 0:2])
                    elif last_batch and f % 4 == 2:
                        nc.scalar.dma_start(H[ci, f // 4, :, 2:3], h4[:, 2:3])
                    elif last_batch and f % 4 == 3:
                        nc.sync.dma_start(H[ci, f // 4, :, 3:4], h4[:, 3:4])
                    elif f % 4 == 3:
                        nc.scalar.dma_start(H[ci, f // 4], h4[:])
    nc.compile()
    return nc


def build_l2(D, TPC, unit_nw, eps=1e-6):
    """Per-core combine + output proj + residual + RMS norm.

    Y[t, j] = nw[j] * (XIN[t,j] + sum_k CT[k,t]*Wo[k,j]) / rms(t)
    CT = g1*A + g2*B (host-combined, bf16); XIN = x_shard + bo (fp32).
    Y output is bf16 (host upcasts).

    The last n-chunk runs m-outer so each m's epilogue (RMS + scale +
    store) chains behind its own k-loop and overlaps the next m's
    matmuls; only m=M-1's epilogue trails the final matmul.

    unit_nw=True specializes norm_w == 1 (scale-by-rstd runs as a scalar
    engine activation; multiplying by 1 is exact) so the vector engine
    stays under the per-m tensor budget during the epilogue phase.
    """
    K = D // P
    M = TPC // P
    NC4 = D // 512
    KB = K // 4          # k-tiles bundled per DMA
    KB8 = 1              # leading k-bundles in fp8 (4 k-tiles = 2 DR pairs)
    KBF = KB - KB8       # remaining bf16 k-bundles
    KF = (K - 4 * KB8)   # bf16 k-tiles
    nc = bacc.Bacc("TRN2", target_bir_lowering=False, debug=False)
    # CT/WO are host-pretiled so every DMA reads long contiguous
    # per-partition runs — strided reads from the natural [D, x] layout
    # only sustain ~40% of DMA bandwidth and gate the lead-in.
    # The first 4 k-tiles run as fp8e4 DoubleRow matmuls (CT8/WO8),
    # accumulating into the same PSUM group as the bf16 remainder; all Wo
    # slices are prescaled x64 on host (exact in bf16) and the 1/64 is
    # folded into the psum-eviction op.  This shaves ~12% off L2's PE time
    # for ~0.2e-2 of extra rel err.
    # DMA discipline: dma_start costs ~0.6us serial config time on the
    # issuing engine, so transfers are few and large.  CT + XIN ride the
    # SP (sync) queue, WO rides the Activation queue (its prelude configs
    # precede all Act compute in program order).
    CT8 = nc.dram_tensor("CT8", [P, 2, 2, TPC], F8, kind="ExternalInput")
    CT = nc.dram_tensor("CT", [KBF, P, 4, TPC], BF, kind="ExternalInput")
    XIN = nc.dram_tensor("XIN", [TPC, D], BF, kind="ExternalInput")
    WO8 = nc.dram_tensor("WO8", [NC4, P, 2, 2, 512], F8, kind="ExternalInput")
    WO = nc.dram_tensor("WO", [NC4, KBF, P, 4, 512], BF, kind="ExternalInput")
    NW = nc.dram_tensor("NW", [D], F32, kind="ExternalInput")
    Y = nc.dram_tensor("Y", [TPC, D], BF, kind="ExternalOutput")

    XINr = XIN[:, :].rearrange("(m p) d -> p m d", p=P)
    DR = mybir.MatmulPerfMode.DoubleRow
    EV = 1.0 / WSCALE    # psum carries x64 products

    with tile.TileContext(nc) as tc:
        with (
            tc.tile_pool(name="consts", bufs=1) as consts,
            tc.tile_pool(name="ct", bufs=1) as ctp,
            tc.tile_pool(name="wo", bufs=3) as wop,
            tc.tile_pool(name="yall", bufs=1) as yallp,
            tc.tile_pool(name="sq", bufs=3) as sqp,
            tc.tile_pool(name="xin", bufs=2) as xinp,
            tc.tile_pool(name="yn", bufs=2) as ynp,
            tc.tile_pool(name="ssm", bufs=1) as ssmp,
            tc.tile_pool(name="stat", bufs=4) as statp,
            tc.tile_pool(name="ps", bufs=1, space="PSUM") as psp,
        ):
            # CT as one SBUF-resident tile; first bundle k-split so the
            # n=0 k-loop starts after ~0.4MB.
            # fp8 head is small (0.5MB CT8 + 0.25MB/chunk WO8): the first
            # matmuls gate on it; bf16 CT bundles stream behind on sync,
            # CT's last bundle rides the scalar queue between wo-n0 and
            # wo-n1 so both queues run flat out through the n=0 k-loop.
            ct8_all = ctp.tile([P, 2, 2, TPC], F8, tag="ct8", name="ct8")
            ct_all = ctp.tile([P, KBF, 4, TPC], BF, tag="ctbf", name="ctbf")
            WOr = [WO[n].rearrange("b p j x -> p b j x") for n in range(NC4)]
            wo8_cur = wop.tile([P, 2, 2, 512], F8, tag="wo8", name="wo8")
            wo_cur = wop.tile([P, KBF, 4, 512], BF, tag="wo", name="wo")
            # sync: fp8 head k-pair 0, then the bf16 CT bundles in
            # consumption order; scalar: WO n0 + fp8 head k-pair 1 + CT's
            # last bundle + WO n1 -- both queues ~paced to the n=0 k-loop.
            nc.sync.dma_start(ct8_all[:, 0], CT8[:, 0])
            nc.scalar.dma_start(wo8_cur[:], WO8[0])
            nc.scalar.dma_start(ct8_all[:, 1], CT8[:, 1])
            nc.sync.dma_start(ct_all[:, 0, 0:2], CT[0, :, 0:2])
            nc.scalar.dma_start(wo_cur[:, 0:1], WOr[0][:, 0:1])
            nc.sync.dma_start(ct_all[:, 0, 2:4], CT[0, :, 2:4])
            nc.scalar.dma_start(wo_cur[:, 1:KBF], WOr[0][:, 1:KBF])
            nc.sync.dma_start(ct_all[:, 1], CT[1])
            nc.scalar.dma_start(ct_all[:, 2], CT[2])
            wo8_nxt = wop.tile([P, 2, 2, 512], F8, tag="wo8", name="wo8")
            wo_nxt = wop.tile([P, KBF, 4, 512], BF, tag="wo", name="wo")
            nc.scalar.dma_start(wo8_nxt[:], WO8[1])
            nc.scalar.dma_start(wo_nxt[:], WOr[1])
            # y_all accumulator (fp32); the residual XIN streams in as
            # per-(m,n) bf16 slices added at psum-eviction time, so its
            # bytes never compete with the lead-in wo/ct stream.
            y_all = yallp.tile([P, M, D], F32)
            nw_sb = None
            if not unit_nw:
                nw_sb = consts.tile([P, D], F32)
                nwap = NW[:]
                nw_bcast = bass.AP(
                    tensor=nwap.tensor, offset=nwap.offset, ap=[[0, P]] + list(nwap.ap)
                )
                nc.sync.dma_start(nw_sb[:], nw_bcast)
            eps_sb = consts.tile([P, 1], F32)
            nc.vector.memset(eps_sb[:], eps)

            ssm_t = ssmp.tile([P, M], F32)
            ss_m = [ssm_t[:, m : m + 1] for m in range(M)]

            def stats(m, n, y_slice, first):
                # incremental RMS stats: ss_m[m] += sum(y_slice^2)
                ncols = y_slice.shape[-1]
                sq = sqp.tile([P, 1024], F32, tag="sq", name="sq")
                ssp = statp.tile([P, 1], F32, tag="ssp", name="ssp")
                nc.scalar.activation(
                    sq[:, :ncols],
                    y_slice,
                    mybir.ActivationFunctionType.Square,
                    accum_out=ssp[:],
                )
                if first:
                    nc.vector.tensor_copy(ss_m[m], ssp[:])
                else:
                    nc.vector.tensor_add(ss_m[m], ss_m[m], ssp[:])

            def epilogue(m):
                y_m = y_all[:, m, :]
                rms = statp.tile([P, 1], F32, tag="rms", name="rms")
                nc.scalar.activation(
                    rms[:],
                    ss_m[m],
                    mybir.ActivationFunctionType.Sqrt,
                    bias=eps_sb[:],
                    scale=1.0 / D,
                )
                rstd = statp.tile([P, 1], F32, tag="rstd", name="rstd")
                nc.vector.reciprocal(rstd[:], rms[:])
                yn = ynp.tile([P, D], BF, tag="yn", name="yn")
                if unit_nw and m == M - 1:
                    # final m: its scale trails the last matmul; pipeline
                    # 512-col pieces across scalar+vector and both HWDGE
                    # queues so the last Y write starts ASAP
                    for q in range(4):
                        qs = slice(q * 512, (q + 1) * 512)
                        if q % 2 == 0:
                            nc.scalar.activation(
                                yn[:, qs],
                                y_all[:, m, qs],
                                mybir.ActivationFunctionType.Identity,
                                bias=0.0,
                                scale=rstd[:],
                            )
                        else:
                            nc.vector.tensor_scalar_mul(
                                yn[:, qs], y_all[:, m, qs], rstd[:]
                            )
                        eng = nc.sync if q % 2 == 0 else nc.scalar
                        eng.dma_start(Y[m * P : (m + 1) * P, qs], yn[:, qs])
                    return
                for h in range(1):
                    hs = slice(0, D)
                    if unit_nw:
                        # alternate scale engines by m: vector also carries
                        # the evicts/recips, scalar the squares/sqrts, so
                        # neither backlogs behind the 3.0us/m tensor pace
                        if m % 2 == 1:
                            nc.scalar.activation(
                                yn[:, hs],
                                y_all[:, m, hs],
                                mybir.ActivationFunctionType.Identity,
                                bias=0.0,
                                scale=rstd[:],
                            )
                        else:
                            nc.vector.tensor_scalar_mul(
                                yn[:, hs], y_all[:, m, hs], rstd[:]
                            )
                    else:
                        nc.vector.scalar_tensor_tensor(
                            yn[:, hs],
                            y_all[:, m, hs],
                            rstd[:],
                            nw_sb[:, hs],
                            op0=mybir.AluOpType.mult,
                            op1=mybir.AluOpType.mult,
                        )
                    nc.sync.dma_start(Y[m * P : (m + 1) * P, hs], yn[:, hs])

            pss = [psp.tile([P, 512], F32, tag=f"ps{m}", name=f"ps{m}")
                   for m in range(M)]

            # PE warm-up during the DMA lead-in (see build_l1)
            warm = consts.tile([P, 64], BF)
            nc.vector.memset(warm[:], 1.0)
            for _ in range(36):
                nc.tensor.matmul(pss[0][0:64, 0:64], lhsT=warm[:, 0:64],
                                 rhs=warm[:], start=True, stop=True)

            xin_prev = None
            for n in range(NC4):
                n0 = n * 512
                # one batched XIN load per n-chunk
                xin_n = xinp.tile([P, M, 512], BF, tag="xin", name="xin")
                nc.sync.dma_start(xin_n[:], XINr[:, :, n0 : n0 + 512])
                if n + 2 < NC4:
                    wo8_n2 = wop.tile([P, 2, 2, 512], F8, tag="wo8", name="wo8")
                    wo_n2 = wop.tile([P, KBF, 4, 512], BF, tag="wo", name="wo")
                    nc.scalar.dma_start(wo8_n2[:], WO8[n + 2])
                    nc.scalar.dma_start(wo_n2[:], WOr[n + 2])
                def mm_f8(m, kp, start):
                    nc.tensor.matmul(
                        pss[m][:],
                        lhsT=ct8_all[:, kp, :, m * P : (m + 1) * P],
                        rhs=wo8_cur[:, kp],
                        start=start,
                        stop=False,
                        perf_mode=DR,
                    )

                def mm_bf(m, k, stop):
                    nc.tensor.matmul(
                        pss[m][:],
                        lhsT=ct_all[:, k // 4, k % 4, m * P : (m + 1) * P],
                        rhs=wo_cur[:, k // 4, k % 4],
                        start=False,
                        stop=stop,
                    )

                if n == 0:
                    # k-outer for the first chunk: consumes each (wo, ct)
                    # bundle over 8 matmuls, pacing the k-loop to the DMA
                    # stream instead of stalling m=0 on the full 6MB.
                    for kp in range(2):
                        for m in range(M):
                            mm_f8(m, kp, kp == 0)
                    for k in range(KF):
                        for m in range(M):
                            mm_bf(m, k, k == KF - 1)
                    for m in range(M):
                        # defer the n=0 residual add: evict psum (x64) with
                        # a scaled copy so nothing here waits on XIN
                        nc.vector.tensor_scalar_mul(y_all[:, m, 0:512],
                                                    pss[m][:], EV)
                else:
                    # m-outer, k-inner: 16 consecutive matmuls accumulate
                    # into one PSUM bank before it's read (avoids psum-queue
                    # depth-cycling micro-idles).
                    for m in range(M):
                        for kp in range(2):
                            mm_f8(m, kp, kp == 0)
                        for k in range(KF):
                            mm_bf(m, k, k == KF - 1)
                        y_slice = y_all[:, m, n0 : n0 + 512]
                        # y = psum/64 + xin
                        nc.vector.scalar_tensor_tensor(
                            y_slice, pss[m][:], EV, xin_n[:, m],
                            op0=mybir.AluOpType.mult,
                            op1=mybir.AluOpType.add,
                        )
                        if n == 1:
                            # catch up n=0's deferred residual add and run
                            # stats over both chunks at once
                            nc.vector.tensor_add(
                                y_all[:, m, 0:512], y_all[:, m, 0:512],
                                xin_prev[:, m],
                            )
                            stats(m, n, y_all[:, m, 0:1024], first=True)
                        else:
                            stats(m, n, y_slice, first=False)
                        if n + 1 == NC4:
                            # chain each m's epilogue behind its own k-loop
                            # so only m=M-1's trails the final matmul
                            epilogue(m)
                xin_prev = xin_n
                if n + 1 < NC4:
                    wo_cur = wo_nxt
                    wo8_cur = wo8_nxt
                    if n + 2 < NC4:
                        wo_nxt = wo_n2
                        wo8_nxt = wo8_n2
    nc.compile()
    return nc


def host_dispatch(xf, Wr, br):
    """Router + top-2 + softmax gates + expert grouping. Returns dispatch info."""
    T, D = xf.shape
    E = Wr.shape[1]
    logits = xf @ Wr + br
    i1 = np.argmax(logits, axis=1)
    l2 = logits.copy()
    l2[np.arange(T), i1] = -np.inf
    i2 = np.argmax(l2, axis=1)
    v1 = logits[np.arange(T), i1]
    v2 = logits[np.arange(T), i2]
    e2 = np.exp(v2 - v1)
    g1 = (1.0 / (1.0 + e2)).astype(np.float32)
    g2 = (e2 / (1.0 + e2)).astype(np.float32)

    # flat slots (t,s) grouped by expert, stable by (token, slot)
    ee = np.stack([i1, i2], 1).ravel()          # [2T]
    gg = np.stack([g1, g2], 1).ravel()
    tt = np.repeat(np.arange(T), 2)
    order = np.argsort(ee, kind="stable")
    counts = np.bincount(ee, minlength=E)
    starts = np.concatenate([[0], np.cumsum(counts)[:-1]])
    rank = np.empty(2 * T, np.int64)
    rank[order] = np.arange(2 * T)
    pos = rank - starts[ee]                      # position within expert's list
    return dict(
        e1=i1, e2=i2, counts=counts, order=order, starts=starts,
        pos=pos.reshape(T, 2), tok=tt, gate=gg, g1=g1, g2=g2,
    )


def prep_l1_inputs(xf, d, We, be):
    """Per-expert L1 inputs: gathered+pretiled fp8 XT, fp8 W (x WSCALE),
    fp32 be.  Tokens past CAP (expert overflow) are evaluated here on host
    in fp32 and stashed in d["Hov"] for the combine.
    """
    T, D = xf.shape
    E = We.shape[0]
    K2 = D // P // 2
    F = D // P
    C = CAP // 512
    counts = d["counts"]
    We_f = np.asarray(We, np.float32)
    be_f = np.asarray(be, np.float32)
    in1 = []
    Hov = []
    for e in range(E):
        n_e = int(counts[e])
        sel = d["order"][d["starts"][e] : d["starts"][e] + n_e]
        toks = d["tok"][sel]
        n_dev = min(n_e, CAP)
        Xg = np.zeros((CAP, D), np.float32)
        Xg[:n_dev] = xf[toks[:n_dev]]
        # [C, P, K2, 2, n]: DoubleRow k-pair groups, contiguous 8KB
        # per-partition DMA runs
        XT_T = np.ascontiguousarray(
            Xg.astype(F8NP).reshape(C, 512, K2, 2, P).transpose(0, 4, 2, 3, 1)
        )
        # [P, F, K2, 2, P]: partition-major so f-tile slabs DMA as single
        # transfers with (4*f_span)KB per-partition runs
        W_T = np.ascontiguousarray(
            (We_f[e] * WSCALE).astype(F8NP)
            .reshape(K2, 2, P, F, P).transpose(2, 3, 0, 1, 4)
        )
        in1.append({"XT": XT_T, "W": W_T,
                    "BE": np.ascontiguousarray(be_f[e].reshape(F, P).T)})
        if n_e > CAP:
            Xov = xf[toks[CAP:]]                        # [m, D] fp32
            pre = Xov @ We_f[e] + be_f[e]
            ho = (pre / (1.0 + np.exp(-pre))).astype(np.float32)
            Hov.append(ho.T)                            # [D, m]
        else:
            Hov.append(np.zeros((D, 0), np.float32))
    d["Hov"] = Hov
    return in1, CAP


def prep_l2_inputs(xf, d, H, Wo, bo, norm_w):
    """Per-core L2 inputs. CT = g1*A + g2*B combined on host (fp32 math);
    the first 512 k-rows go out as fp8 (DoubleRow pairs), the rest bf16.
    All Wo slices are prescaled x64 (exact in bf16; lifts the fp8 rows out
    of e4m3's subnormal range).  XIN = x + bo in fp32."""
    T, D = xf.shape
    TPC = T // NCORE
    KB = D // P // 4
    KBF = KB - 1
    NC4 = D // 512
    Wo_f = np.asarray(Wo, np.float32) * WSCALE
    # fp8 head: [NC4, P, 2, 2, 512] DoubleRow k-pair groups
    Wo8_b = np.ascontiguousarray(
        Wo_f[:512].astype(F8NP)
        .reshape(2, 2, P, NC4, 512).transpose(3, 2, 0, 1, 4)
    )
    # bf16 tail: [NC4, KBF, P, 4, 512]
    Wo_b = np.ascontiguousarray(
        Wo_f[512:].astype(BF16)
        .reshape(KBF, 4, P, NC4, 512).transpose(3, 0, 2, 1, 4)
    )
    bo_f = np.asarray(bo, np.float32)
    nw_f = np.asarray(norm_w, np.float32)
    e1, e2, pos = d["e1"], d["e2"], d["pos"]
    g1, g2 = d["g1"], d["g2"]
    # device H (raw [C, FQ, P, 4, 512] batches -> [D, CAP], first CAP
    # slots) + host-computed overflow columns
    def unpack_h(h_raw):
        return np.ascontiguousarray(
            np.asarray(h_raw, np.float32).transpose(1, 3, 2, 0, 4).reshape(D, CAP)
        )
    Hfull = [np.concatenate([unpack_h(H[e]), d["Hov"][e]], axis=1)
             for e in range(H.shape[0])]
    in2 = []
    for c in range(NCORE):
        tl = np.arange(c * TPC, (c + 1) * TPC)
        CTf = np.empty((D, TPC), np.float32)
        BTf = np.empty((D, TPC), np.float32)
        for e in range(H.shape[0]):
            s1 = e1[tl] == e
            if s1.any():
                CTf[:, s1] = Hfull[e][:, pos[tl[s1], 0]]
            s2 = e2[tl] == e
            if s2.any():
                BTf[:, s2] = Hfull[e][:, pos[tl[s2], 1]]
        CTf = CTf * g1[tl][None, :] + BTf * g2[tl][None, :]
        CT8t = np.ascontiguousarray(
            CTf[:512].astype(F8NP).reshape(2, 2, P, TPC).transpose(2, 0, 1, 3)
        )
        CTt = np.ascontiguousarray(
            CTf[512:].reshape(KBF, 4, P, TPC).transpose(0, 2, 1, 3)
        ).astype(BF16)
        XIN = (xf[tl] + bo_f[None, :]).astype(BF16)
        in2.append({"CT8": CT8t, "CT": CTt, "XIN": XIN,
                    "WO8": Wo8_b, "WO": Wo_b, "NW": nw_f})
    return in2


# ----------------------------------------------------------------------------
# Harness entry point: full (unsharded) inputs -> full output.
# ----------------------------------------------------------------------------
_L1_CACHE = {}
_L2_CACHE = {}


def kernel(x, Wr, br, We, be, Wo, bo, norm_w):
    B, S, D = x.shape
    E = We.shape[0]
    T = B * S
    TPC = T // NCORE
    xf = np.ascontiguousarray(np.asarray(x, np.float32).reshape(T, D))
    d = host_dispatch(xf, np.asarray(Wr, np.float32), np.asarray(br, np.float32))

    in1, Bcap = prep_l1_inputs(xf, d, We, be)
    if (D, Bcap) not in _L1_CACHE:
        _L1_CACHE[(D, Bcap)] = build_l1(D, Bcap)
    r1 = run_bass_kernel_spmd(_L1_CACHE[(D, Bcap)], in1, list(range(NCORE)))
    H = np.stack([r1.results[e]["H"] for e in range(E)])

    in2 = prep_l2_inputs(xf, d, H, Wo, bo, norm_w)
    unit_nw = bool(np.all(np.asarray(norm_w, np.float32) == 1.0))
    if (D, TPC, unit_nw) not in _L2_CACHE:
        _L2_CACHE[(D, TPC, unit_nw)] = build_l2(D, TPC, unit_nw)
    r2 = run_bass_kernel_spmd(_L2_CACHE[(D, TPC, unit_nw)], in2, list(range(NCORE)))
    Y = np.concatenate([r2.results[c]["Y"] for c in range(NCORE)], axis=0)
    return Y.reshape(B, S, D).astype(np.asarray(x).dtype)

